# revision 1
# baseline (speedup 1.0000x reference)
"""3-layer GCN node predictor on 8 Trainium2 NeuronCores (Bass/Tile SPMD).

Strategy (graph/data parallel, per sharding hint):
- Nodes sharded into 8 contiguous chunks (12544 padded rows per core); each
  core aggregates the in-edges of its own dst nodes.
- Per layer, the gather table T_L = o_{L-1} @ W_L ([100352, 64] fp32, 256B
  rows) is built shard-wise and AllGathered to every core's DRAM.
- Per-edge gather of T_L[src] uses gpsimd dma_gather (int16 indices ->
  4 table quarters of 25088 rows), round-robin over 4 SWDGE queues.
- Scatter-add uses TensorE: one-hot S [128 edges, 64 dst] built on DVE via
  is_equal against an iota row, matmul S.T @ msg accumulated in PSUM.
- Self loops are applied node-wise from the SBUF-resident own chunk.
"""
import numpy as np

import concourse.bass as bass
import concourse.bacc as bacc
import concourse.tile as tile
import concourse.mybir as mybir
from concourse.bass_utils import run_bass_kernel_spmd

NCORES = 8
N = 100000
E = 3200000
F_IN = 128
HID = 32
NCLS = 10
RC = 12500          # real nodes per core
PC = 12544          # padded nodes per core (98 * 128)
NP = PC * NCORES    # padded total nodes (100352)
Q4 = NP // 4        # table quarter rows (25088), int16-addressable
ELEM = 64           # table row elements (256B rows)
W = 64              # dst window
NWIN = PC // W      # 196 windows per core
SGW = 4             # windows per supergroup
NSG = NWIN // SGW   # 49
NG = PC // 128      # 98 node groups of 128
EPS2 = 1e-24

_cache = {}


def _host_prep(x, edge_index, edge_weights):
    src = np.asarray(edge_index[0], dtype=np.int64)
    dst = np.asarray(edge_index[1], dtype=np.int64)
    ew = np.asarray(edge_weights, dtype=np.float64)

    deg = np.bincount(dst, weights=ew, minlength=N) + 1.0
    dinv = np.where(deg > 0, 1.0 / np.sqrt(deg), 0.0)
    cnorm_e = (dinv[src] * ew * dinv[dst]).astype(np.float32)
    s2 = (dinv * dinv).astype(np.float32)

    psrc = (src // RC) * PC + (src % RC)          # padded global src ids

    per_core = []
    for c in range(NCORES):
        m = (dst >= RC * c) & (dst < RC * (c + 1))
        es = psrc[m]
        ed = dst[m] - RC * c
        en = cnorm_e[m]
        w_id = ed // W
        q_id = es // Q4
        order = np.lexsort((ed, q_id, w_id))      # sort by (w, q, dst)
        per_core.append((es[order], ed[order], en[order],
                         w_id[order], q_id[order]))

    # per (w, q) counts and max over cores
    counts = np.zeros((NCORES, NWIN, 4), dtype=np.int64)
    for c in range(NCORES):
        _, _, _, w_id, q_id = per_core[c]
        np.add.at(counts[c], (w_id, q_id), 1)
    cmax = counts.max(axis=0)
    t_wq = (cmax + 127) // 128                    # tiles per (w, q)
    for w_i in range(NWIN):
        if t_wq[w_i].sum() == 0:
            t_wq[w_i, 0] = 1

    # global tile order: (sg, q, w, k)
    tile_of = {}
    T_total = 0
    call_meta = []                                # (sg, q, t0, ntiles)
    for sg in range(NSG):
        for q in range(4):
            t0 = T_total
            for w_i in range(sg * SGW, (sg + 1) * SGW):
                for k in range(t_wq[w_i, q]):
                    tile_of[(w_i, q, k)] = T_total
                    T_total += 1
            call_meta.append((sg, q, t0, T_total - t0))

    # slot arrays
    idx16 = np.zeros((T_total * 128,), dtype=np.int16)
    cnorm = np.zeros((T_total * 128,), dtype=np.float32)
    dstrel = np.full((T_total * 128,), -1.0, dtype=np.float32)
    idx16_all = np.zeros((NCORES, T_total * 128), dtype=np.int16)
    cnorm_all = np.zeros((NCORES, T_total * 128), dtype=np.float32)
    dstrel_all = np.full((NCORES, T_total * 128), -1.0, dtype=np.float32)
    for c in range(NCORES):
        es, ed, en, w_id, q_id = per_core[c]
        # position within (w, q) run
        keys = w_id * 4 + q_id
        # edges already sorted by (w, q); rank within group:
        boundaries = np.flatnonzero(np.diff(keys, prepend=-1))
        ranks = np.arange(len(keys)) - np.repeat(boundaries, np.diff(np.append(boundaries, len(keys))))
        k_tile = ranks // 128
        k_part = ranks % 128
        gtile = np.array([tile_of[(w, q, k)] for (w, q, k) in zip(w_id, q_id, k_tile)])
        slot = gtile * 128 + k_part
        idx16_all[c, slot] = (es % Q4).astype(np.int16)
        cnorm_all[c, slot] = en
        dstrel_all[c, slot] = (ed - w_id * W).astype(np.float32)

    # device layouts
    # cnorm/dstrel resident [128, T]: flat p * T + t; slot = t*128 + p
    def to_pt(a):
        return np.ascontiguousarray(a.reshape(-1, T_total, 128).transpose(0, 2, 1)).reshape(NCORES, -1)

    cnorm_pt = to_pt(cnorm_all)
    dstrel_pt = to_pt(dstrel_all)

    # idx per call: wrapped [128, 8*ntiles] int16, idx j of call at [j%16, j//16],
    # replicated across the 8 groups of 16 partitions. Flattened per call.
    idx_blocks = np.zeros((NCORES, T_total * 1024), dtype=np.int16)
    for (sg, q, t0, nt) in call_meta:
        if nt == 0:
            continue
        nidx = nt * 128
        for c in range(NCORES):
            blk = idx16_all[c, t0 * 128:(t0 + nt) * 128]
            wrp = blk.reshape(nidx // 16, 16).T              # [16, nidx/16]
            rep = np.tile(wrp, (8, 1))                       # [128, nidx/16]
            idx_blocks[c, t0 * 1024:t0 * 1024 + nidx * 8] = rep.reshape(-1)

    # s2/dinv resident [128, NG]: flat p * NG + g ; node g*128+p
    def node_pt(v):
        pad = np.zeros((NCORES, PC), dtype=np.float32)
        for c in range(NCORES):
            pad[c, :RC] = v[RC * c:RC * (c + 1)]
        return np.ascontiguousarray(pad.reshape(NCORES, NG, 128).transpose(0, 2, 1)).reshape(NCORES, -1)

    s2_pt = node_pt(s2)
    dinv_pt = node_pt(dinv.astype(np.float32))
    ones_fast = bool(np.all(np.asarray(edge_weights) == 1.0))

    # x chunks
    x = np.asarray(x, dtype=np.float32)
    x_pad = np.zeros((NCORES, PC, F_IN), dtype=np.float32)
    for c in range(NCORES):
        x_pad[c, :RC] = x[RC * c:RC * (c + 1)]

    return dict(
        T_total=T_total, t_wq=t_wq, tile_of=tile_of, call_meta=call_meta,
        cnorm_pt=cnorm_pt, dstrel_pt=dstrel_pt, idx_blocks=idx_blocks,
        s2_pt=s2_pt, dinv_pt=dinv_pt, ones_fast=ones_fast, x_pad=x_pad,
    )


def _build_program(meta, reps=1, skip_gather=False, skip_scatter=False, ones_fast=False, debug_o1=False, dbgL=0):
    T_total = meta["T_total"]
    t_wq = meta["t_wq"]
    tile_of = meta["tile_of"]
    call_meta = meta["call_meta"]
    f32 = mybir.dt.float32

    nc = bacc.Bacc("TRN2", target_bir_lowering=False, debug=False,
                   num_devices=NCORES, num_swdge_queues=4)

    t_x = nc.dram_tensor("x_c", [PC * F_IN], f32, kind="ExternalInput").ap()
    t_idx = nc.dram_tensor("idxb", [T_total * 1024], mybir.dt.int16, kind="ExternalInput").ap()
    t_cnorm = nc.dram_tensor("cnorm", [128 * T_total], f32, kind="ExternalInput").ap()
    t_dstrel = nc.dram_tensor("dstrel", [128 * T_total], f32, kind="ExternalInput").ap()
    t_s2 = nc.dram_tensor("s2", [128 * NG], f32, kind="ExternalInput").ap()
    t_dinv = nc.dram_tensor("dinv", [128 * NG], f32, kind="ExternalInput").ap()
    t_w1 = nc.dram_tensor("w1", [F_IN, HID], f32, kind="ExternalInput").ap()
    t_w2 = nc.dram_tensor("w2", [HID, HID], f32, kind="ExternalInput").ap()
    t_w3 = nc.dram_tensor("w3", [HID, HID], f32, kind="ExternalInput").ap()
    t_wl = nc.dram_tensor("wl", [3 * HID, NCLS], f32, kind="ExternalInput").ap()
    t_brep = nc.dram_tensor("brep", [3, 128, HID], f32, kind="ExternalInput").ap()
    t_blrep = nc.dram_tensor("blrep", [128, NCLS], f32, kind="ExternalInput").ap()
    t_iota = nc.dram_tensor("iota", [128, W], f32, kind="ExternalInput").ap()
    t_eye = nc.dram_tensor("eye", [128, 128], f32, kind="ExternalInput").ap()
    t_y = nc.dram_tensor("y", [PC * NCLS], f32, kind="ExternalOutput").ap()

    with tile.TileContext(nc) as tc:
        with (
            tc.tile_pool(name="const", bufs=1) as cp,
            tc.tile_pool(name="resident", bufs=1) as rp,
            tc.tile_pool(name="work", bufs=3) as wp,
            tc.tile_pool(name="msgp", bufs=2) as mp,
            tc.tile_pool(name="sp", bufs=6) as spool,
            tc.tile_pool(name="psum", bufs=2, space="PSUM") as pp,
            tc.tile_pool(name="psum2", bufs=2, space="PSUM") as pp2,
            tc.tile_pool(name="dram", bufs=1, space="DRAM") as dp,
        ):
            # ---- constants / residents ----
            w1_t = cp.tile([F_IN, HID], f32); nc.sync.dma_start(out=w1_t[:], in_=t_w1[:])
            w2_t = cp.tile([HID, HID], f32); nc.sync.dma_start(out=w2_t[:], in_=t_w2[:])
            w3_t = cp.tile([HID, HID], f32); nc.sync.dma_start(out=w3_t[:], in_=t_w3[:])
            wl_ts = []
            for L in range(3):
                wt = cp.tile([HID, NCLS], f32, tag=f"wl{L}", name=f"wl{L}")
                nc.sync.dma_start(out=wt[:], in_=t_wl[HID * L:HID * (L + 1), :])
                wl_ts.append(wt)
            brep_t = cp.tile([128, 3, HID], f32)
            nc.sync.dma_start(out=brep_t[:], in_=t_brep.rearrange("l p h -> p l h"))
            blrep_t = cp.tile([128, NCLS], f32); nc.sync.dma_start(out=blrep_t[:], in_=t_blrep[:])
            iota_t = cp.tile([128, W], f32); nc.sync.dma_start(out=iota_t[:], in_=t_iota[:])
            eye_t = cp.tile([128, 128], f32); nc.sync.dma_start(out=eye_t[:], in_=t_eye[:])
            cnorm_t = rp.tile([128, T_total], f32)
            nc.sync.dma_start(out=cnorm_t[:], in_=t_cnorm.rearrange("(p t) -> p t", t=T_total))
            dstrel_t = rp.tile([128, T_total], f32)
            nc.sync.dma_start(out=dstrel_t[:], in_=t_dstrel.rearrange("(p t) -> p t", t=T_total))
            s2_t = rp.tile([128, NG], f32)
            nc.sync.dma_start(out=s2_t[:], in_=t_s2.rearrange("(p g) -> p g", g=NG))
            dinv_t = rp.tile([128, NG], f32)
            nc.sync.dma_start(out=dinv_t[:], in_=t_dinv.rearrange("(p g) -> p g", g=NG))

            h_own = rp.tile([128, NG, HID], f32)          # own chunk of current table (pre-pad)
            o_bufs = [rp.tile([128, NG, HID], f32, tag=f"o{L}", name=f"o{L}") for L in range(3)]

            tables = [dp.tile([NP * ELEM], f32, tag=f"table{L}", name=f"table{L}") for L in range(3)]
            in_bs = [dp.tile([PC * ELEM], f32, tag=f"inb{L}", name=f"inb{L}") for L in range(3)]

            for _rep in range(reps):
                x_v = t_x.rearrange("(g p f) -> g p f", p=128, f=F_IN)

                def chain_write(L, g, o_ap):
                    """From o tile [128, HID] compute h = o @ W_{L+1}, write padded
                    row block to in_bs[L+1] and h_own."""
                    wn = [None, w2_t, w3_t][L + 1]
                    ot_ps = pp2.tile([HID, 128], f32, tag="tps")
                    nc.tensor.transpose(out=ot_ps[:], in_=o_ap, identity=eye_t[:])
                    ot_sb = wp.tile([HID, 128], f32, tag="otsb")
                    nc.vector.tensor_copy(out=ot_sb[:], in_=ot_ps[:])
                    h_ps = pp.tile([128, HID], f32, tag="hps")
                    nc.tensor.matmul(h_ps[:], lhsT=ot_sb[:], rhs=wn[:], start=True, stop=True)
                    h64 = wp.tile([128, ELEM], f32, tag="h64")
                    nc.vector.memset(h64[:, HID:], 0.0)
                    if ones_fast:
                        nc.vector.tensor_scalar(out=h64[:, :HID], in0=h_ps[:],
                                                scalar1=dinv_t[:, g:g + 1], scalar2=None,
                                                op0=mybir.AluOpType.mult)
                    else:
                        nc.vector.tensor_copy(out=h64[:, :HID], in_=h_ps[:])
                    nc.vector.tensor_copy(out=h_own[:, g, :], in_=h64[:, :HID])
                    nc.sync.dma_start(
                        out=in_bs[L + 1][:].rearrange("(g p e) -> g p e", p=128, e=ELEM)[g],
                        in_=h64[:])

                # ---- layer 1 table: h1 = x @ W1 ----
                for g in range(NG):
                    xt = wp.tile([128, F_IN], f32, tag="xt")
                    nc.sync.dma_start(out=xt[:], in_=x_v[g])
                    xT_ps = pp2.tile([128, 128], f32, tag="tps")
                    nc.tensor.transpose(out=xT_ps[:], in_=xt[:], identity=eye_t[:])
                    xT_sb = wp.tile([128, F_IN], f32, tag="xTsb")
                    nc.vector.tensor_copy(out=xT_sb[:], in_=xT_ps[:])
                    h_ps = pp.tile([128, HID], f32, tag="hps")
                    nc.tensor.matmul(h_ps[:], lhsT=xT_sb[:], rhs=w1_t[:], start=True, stop=True)
                    h64 = wp.tile([128, ELEM], f32, tag="h64")
                    nc.vector.memset(h64[:, HID:], 0.0)
                    if ones_fast:
                        nc.vector.tensor_scalar(out=h64[:, :HID], in0=h_ps[:],
                                                scalar1=dinv_t[:, g:g + 1], scalar2=None,
                                                op0=mybir.AluOpType.mult)
                    else:
                        nc.vector.tensor_copy(out=h64[:, :HID], in_=h_ps[:])
                    nc.vector.tensor_copy(out=h_own[:, g, :], in_=h64[:, :HID])
                    nc.sync.dma_start(
                        out=in_bs[0][:].rearrange("(g p e) -> g p e", p=128, e=ELEM)[g],
                        in_=h64[:])

                # ---- layers ----
                for L in range(3):
                    nc.gpsimd.collective_compute(
                        "AllGather", mybir.AluOpType.bypass,
                        replica_groups=[list(range(NCORES))],
                        ins=[in_bs[L][:]], outs=[tables[L][:]])
                    tab_q = [tables[L][:].rearrange("(n e) -> n e", e=ELEM)[Q4 * q:Q4 * (q + 1)]
                             for q in range(4)]

                    # supergroup tile extents
                    sg_t0 = [min(t0 for (s, q, t0, nt) in call_meta if s == sg)
                             for sg in range(NSG)]
                    sg_t1 = [max(t0 + nt for (s, q, t0, nt) in call_meta if s == sg)
                             for sg in range(NSG)]

                    for sg in range(NSG):
                        t0s, t1s = sg_t0[sg], sg_t1[sg]
                        nts = t1s - t0s
                        msg = mp.tile([128, nts, ELEM], f32, tag="msg")
                        for (s, q, t0, nt) in call_meta:
                            if s != sg or nt == 0:
                                continue
                            nidx = nt * 128
                            idxt = wp.tile([128, nt * 8], mybir.dt.int16, tag="idxt")
                            nc.sync.dma_start(
                                out=idxt[:],
                                in_=t_idx[t0 * 1024:t0 * 1024 + nidx * 8]
                                    .rearrange("(p n) -> p n", p=128))
                            if not skip_gather:
                                nc.gpsimd.dma_gather(
                                    out_ap=msg[:, t0 - t0s:t0 - t0s + nt, :],
                                    in_ap=tab_q[q],
                                    idxs_ap=idxt[:],
                                    num_idxs=nidx, num_idxs_reg=nidx,
                                    elem_size=ELEM, elem_step=ELEM,
                                    single_packet=False, queue_num=q)
                            if not ones_fast:
                                nc.vector.tensor_tensor(
                                    out=msg[:, t0 - t0s:t0 - t0s + nt, :HID],
                                    in0=msg[:, t0 - t0s:t0 - t0s + nt, :HID],
                                    in1=cnorm_t[:, t0:t0 + nt].unsqueeze(-1).to_broadcast([128, nt, HID]),
                                    op=mybir.AluOpType.mult)

                        # two psum groups (2 windows each) per supergroup
                        for half in range(SGW // 2):
                            agg_ps = pp.tile([128, HID], f32, tag="agg")
                            if skip_scatter:
                                nc.vector.memset(agg_ps[:], 0.0)
                            g = sg * 2 + half  # node group index (128 dst)
                            for wi in range(2):
                                w_i = sg * SGW + half * 2 + wi
                                tiles_w = []
                                for q in range(4):
                                    for k in range(t_wq[w_i, q]):
                                        tiles_w.append(tile_of[(w_i, q, k)])
                                # S tiles in runs of up to 8 consecutive
                                runs = []
                                for t in tiles_w:
                                    if runs and runs[-1][0] + runs[-1][1] == t and runs[-1][1] < 8:
                                        runs[-1] = (runs[-1][0], runs[-1][1] + 1)
                                    else:
                                        runs.append((t, 1))
                                s_tiles = {}
                                for (rt, rn) in (runs if not skip_scatter else []):
                                    st = spool.tile([128, 8, W], f32, tag="S")
                                    nc.vector.tensor_tensor(
                                        out=st[:, :rn, :],
                                        in0=dstrel_t[:, rt:rt + rn].unsqueeze(-1).to_broadcast([128, rn, W]),
                                        in1=iota_t[:].unsqueeze(1).to_broadcast([128, rn, W]),
                                        op=mybir.AluOpType.is_equal)
                                    for j in range(rn):
                                        s_tiles[rt + j] = st[:, j, :]
                                for ti, t in (enumerate(tiles_w) if not skip_scatter else []):
                                    nc.tensor.matmul(
                                        agg_ps[64 * wi:64 * (wi + 1), :],
                                        lhsT=s_tiles[t],
                                        rhs=msg[:, t - t0s, :HID],
                                        start=(ti == 0), stop=(ti == len(tiles_w) - 1),
                                        skip_group_check=True)
                            # ---- postproc for node group g ----
                            A = wp.tile([128, HID], f32, tag="A")
                            if ones_fast:
                                # table rows are pre-scaled by dinv[src]; out =
                                # dinv[d]*(agg + h_own) + b
                                tmp = wp.tile([128, HID], f32, tag="tmp")
                                nc.vector.tensor_tensor(out=tmp[:], in0=agg_ps[:],
                                                        in1=h_own[:, g, :],
                                                        op=mybir.AluOpType.add)
                                nc.vector.tensor_scalar(
                                    out=tmp[:], in0=tmp[:], scalar1=dinv_t[:, g:g + 1],
                                    scalar2=None, op0=mybir.AluOpType.mult)
                                nc.vector.tensor_tensor(out=A[:], in0=tmp[:], in1=brep_t[:, L, :],
                                                        op=mybir.AluOpType.add)
                            else:
                                nc.vector.tensor_tensor(out=A[:], in0=agg_ps[:], in1=brep_t[:, L, :],
                                                        op=mybir.AluOpType.add)
                                tmp = wp.tile([128, HID], f32, tag="tmp")
                                nc.vector.tensor_scalar(
                                    out=tmp[:], in0=h_own[:, g, :], scalar1=s2_t[:, g:g + 1],
                                    scalar2=None, op0=mybir.AluOpType.mult)
                                nc.vector.tensor_tensor(out=A[:], in0=A[:], in1=tmp[:],
                                                        op=mybir.AluOpType.add)
                            sq = wp.tile([128, HID], f32, tag="sq")
                            n2 = wp.tile([128, 1], f32, tag="n2")
                            nc.scalar.activation(out=sq[:], in_=A[:],
                                                 func=mybir.ActivationFunctionType.Square,
                                                 accum_out=n2[:])
                            nc.vector.tensor_scalar(out=n2[:], in0=n2[:], scalar1=EPS2,
                                                    scalar2=None, op0=mybir.AluOpType.max)
                            rt_ = wp.tile([128, 1], f32, tag="rt")
                            nc.scalar.activation(out=rt_[:], in_=n2[:],
                                                 func=mybir.ActivationFunctionType.Sqrt)
                            rinv = wp.tile([128, 1], f32, tag="rinv")
                            nc.vector.reciprocal(out=rinv[:], in_=rt_[:])
                            nc.scalar.activation(out=o_bufs[L][:, g, :], in_=A[:],
                                                 func=mybir.ActivationFunctionType.Relu,
                                                 scale=rinv[:])
                            if L < 2:
                                chain_write(L, g, o_bufs[L][:, g, :])

                # ---- head: y = [o1|o2|o3] @ W_lin + b ----
                y_v = t_y.rearrange("(g p k) -> g p k", p=128, k=NCLS)
                for g in range(NG):
                    y_ps = pp.tile([128, NCLS], f32, tag="hps")
                    for L in range(3):
                        ot_ps = pp2.tile([HID, 128], f32, tag="tps")
                        nc.tensor.transpose(out=ot_ps[:], in_=o_bufs[L][:, g, :], identity=eye_t[:])
                        ot_sb = wp.tile([HID, 128], f32, tag="otsb")
                        nc.vector.tensor_copy(out=ot_sb[:], in_=ot_ps[:])
                        nc.tensor.matmul(y_ps[:], lhsT=ot_sb[:], rhs=wl_ts[L][:],
                                         start=(L == 0), stop=(L == 2), skip_group_check=True)
                    y_sb = wp.tile([128, NCLS], f32, tag="ysb")
                    if debug_o1:
                        nc.vector.tensor_copy(out=y_sb[:], in_=o_bufs[dbgL][:, g, :NCLS])
                    else:
                        nc.vector.tensor_tensor(out=y_sb[:], in0=y_ps[:], in1=blrep_t[:],
                                                op=mybir.AluOpType.add)
                    nc.sync.dma_start(out=y_v[g], in_=y_sb[:])

    nc.compile()
    return nc


def _make_in_maps(meta, W1, b1, W2, b2, W3, b3, W_lin, b_lin):
    brep = np.stack([np.tile(np.asarray(b, np.float32)[None, :], (128, 1))
                     for b in (b1, b2, b3)])           # [3,128,HID]
    blrep = np.tile(np.asarray(b_lin, np.float32)[None, :], (128, 1))
    iota = np.tile(np.arange(W, dtype=np.float32)[None, :], (128, 1))
    eye = np.eye(128, dtype=np.float32)
    maps = []
    for c in range(NCORES):
        maps.append({
            "x_c": meta["x_pad"][c].reshape(-1),
            "idxb": meta["idx_blocks"][c],
            "cnorm": meta["cnorm_pt"][c],
            "dstrel": meta["dstrel_pt"][c],
            "s2": meta["s2_pt"][c],
            "dinv": meta["dinv_pt"][c],
            "w1": np.asarray(W1, np.float32), "w2": np.asarray(W2, np.float32),
            "w3": np.asarray(W3, np.float32), "wl": np.asarray(W_lin, np.float32),
            "brep": brep, "blrep": blrep, "iota": iota, "eye": eye,
        })
    return maps


def kernel(x, edge_index, edge_weights, W1, b1, W2, b2, W3, b3, W_lin, b_lin):
    meta = _host_prep(x, edge_index, edge_weights)
    key = ("prog", meta["ones_fast"])
    if key not in _cache:
        _cache[key] = _build_program(meta, ones_fast=meta["ones_fast"])
    nc = _cache[key]
    in_maps = _make_in_maps(meta, W1, b1, W2, b2, W3, b3, W_lin, b_lin)
    res = run_bass_kernel_spmd(nc, in_maps, core_ids=list(range(NCORES)))
    ys = [res.results[c]["y"].reshape(PC, NCLS)[:RC] for c in range(NCORES)]
    return np.concatenate(ys, axis=0).astype(np.float32)



# revision 9
# speedup vs baseline: 1.1346x; 1.1346x over previous
"""3-layer GCN node predictor on 8 Trainium2 NeuronCores (Bass/Tile SPMD).

Strategy (graph/data parallel, per sharding hint):
- Nodes sharded into 8 contiguous chunks (12544 padded rows per core); each
  core aggregates the in-edges of its own dst nodes.
- Per layer, the gather table T_L = o_{L-1} @ W_L ([100352, 64] fp32, 256B
  rows) is built shard-wise and AllGathered to every core's DRAM.
- Per-edge gather of T_L[src] uses gpsimd dma_gather (int16 indices ->
  4 table quarters of 25088 rows), round-robin over 4 SWDGE queues.
- Scatter-add uses TensorE: one-hot S [128 edges, 64 dst] built on DVE via
  is_equal against an iota row, matmul S.T @ msg accumulated in PSUM.
- Self loops are applied node-wise from the SBUF-resident own chunk.
"""
import numpy as np

import concourse.bass as bass
import concourse.bacc as bacc
import concourse.tile as tile
import concourse.mybir as mybir
from concourse.bass_utils import run_bass_kernel_spmd

NCORES = 8
N = 100000
E = 3200000
F_IN = 128
HID = 32
NCLS = 10
RC = 12500          # real nodes per core
PC = 12544          # padded nodes per core (98 * 128)
NP = PC * NCORES    # padded total nodes (100352)
Q4 = NP // 4        # table quarter rows (25088), int16-addressable
ELEM = 64           # table row elements (256B rows)
W = 64              # dst window
NWIN = PC // W      # 196 windows per core
SGW = 4             # windows per supergroup
NSG = NWIN // SGW   # 49
NG = PC // 128      # 98 node groups of 128
EPS2 = 1e-24

_cache = {}


def _host_prep(x, edge_index, edge_weights):
    src = np.asarray(edge_index[0], dtype=np.int64)
    dst = np.asarray(edge_index[1], dtype=np.int64)
    ew = np.asarray(edge_weights, dtype=np.float64)

    deg = np.bincount(dst, weights=ew, minlength=N) + 1.0
    dinv = np.where(deg > 0, 1.0 / np.sqrt(deg), 0.0)
    cnorm_e = (dinv[src] * ew * dinv[dst]).astype(np.float32)
    s2 = (dinv * dinv).astype(np.float32)

    psrc = (src // RC) * PC + (src % RC)          # padded global src ids

    per_core = []
    for c in range(NCORES):
        m = (dst >= RC * c) & (dst < RC * (c + 1))
        es = psrc[m]
        ed = dst[m] - RC * c
        en = cnorm_e[m]
        w_id = ed // W
        q_id = es // Q4
        order = np.lexsort((es, q_id, w_id))      # sort by (w, q, src):
        # src-ascending descriptors within each (w, q) run give the HBM
        # sequential-ish locality; scatter is order-agnostic within a window.
        per_core.append((es[order], ed[order], en[order],
                         w_id[order], q_id[order]))

    # per (w, q) counts and max over cores
    counts = np.zeros((NCORES, NWIN, 4), dtype=np.int64)
    for c in range(NCORES):
        _, _, _, w_id, q_id = per_core[c]
        np.add.at(counts[c], (w_id, q_id), 1)
    cmax = counts.max(axis=0)
    t_wq = (cmax + 127) // 128                    # tiles per (w, q)
    for w_i in range(NWIN):
        if t_wq[w_i].sum() == 0:
            t_wq[w_i, 0] = 1

    # global tile order: (sg, q, w, k)
    tile_of = {}
    T_total = 0
    call_meta = []                                # (sg, q, t0, ntiles)
    for sg in range(NSG):
        for q in range(4):
            t0 = T_total
            for w_i in range(sg * SGW, (sg + 1) * SGW):
                for k in range(t_wq[w_i, q]):
                    tile_of[(w_i, q, k)] = T_total
                    T_total += 1
            call_meta.append((sg, q, t0, T_total - t0))

    # slot arrays
    idx16 = np.zeros((T_total * 128,), dtype=np.int16)
    cnorm = np.zeros((T_total * 128,), dtype=np.float32)
    dstrel = np.full((T_total * 128,), -1.0, dtype=np.float32)
    idx16_all = np.zeros((NCORES, T_total * 128), dtype=np.int16)
    cnorm_all = np.zeros((NCORES, T_total * 128), dtype=np.float32)
    dstrel_all = np.full((NCORES, T_total * 128), -1.0, dtype=np.float32)
    for c in range(NCORES):
        es, ed, en, w_id, q_id = per_core[c]
        # position within (w, q) run
        keys = w_id * 4 + q_id
        # edges already sorted by (w, q); rank within group:
        boundaries = np.flatnonzero(np.diff(keys, prepend=-1))
        ranks = np.arange(len(keys)) - np.repeat(boundaries, np.diff(np.append(boundaries, len(keys))))
        k_tile = ranks // 128
        k_part = ranks % 128
        gtile = np.array([tile_of[(w, q, k)] for (w, q, k) in zip(w_id, q_id, k_tile)])
        slot = gtile * 128 + k_part
        idx16_all[c, slot] = (es % Q4).astype(np.int16)
        cnorm_all[c, slot] = en
        dstrel_all[c, slot] = (ed - w_id * W).astype(np.float32)

    # device layouts
    # cnorm/dstrel resident [128, T]: flat p * T + t; slot = t*128 + p
    def to_pt(a):
        return np.ascontiguousarray(a.reshape(-1, T_total, 128).transpose(0, 2, 1)).reshape(NCORES, -1)

    cnorm_pt = to_pt(cnorm_all)
    dstrel_pt = to_pt(dstrel_all)

    # idx per call: wrapped [128, 8*ntiles] int16, idx j of call at [j%16, j//16],
    # replicated across the 8 groups of 16 partitions. Flattened per call.
    idx_blocks = np.zeros((NCORES, T_total * 1024), dtype=np.int16)
    for (sg, q, t0, nt) in call_meta:
        if nt == 0:
            continue
        nidx = nt * 128
        for c in range(NCORES):
            blk = idx16_all[c, t0 * 128:(t0 + nt) * 128]
            wrp = blk.reshape(nidx // 16, 16).T              # [16, nidx/16]
            rep = np.tile(wrp, (8, 1))                       # [128, nidx/16]
            idx_blocks[c, t0 * 1024:t0 * 1024 + nidx * 8] = rep.reshape(-1)

    # s2/dinv resident [128, NG]: flat p * NG + g ; node g*128+p
    def node_pt(v):
        pad = np.zeros((NCORES, PC), dtype=np.float32)
        for c in range(NCORES):
            pad[c, :RC] = v[RC * c:RC * (c + 1)]
        return np.ascontiguousarray(pad.reshape(NCORES, NG, 128).transpose(0, 2, 1)).reshape(NCORES, -1)

    s2_pt = node_pt(s2)
    dinv_pt = node_pt(dinv.astype(np.float32))
    ones_fast = bool(np.all(np.asarray(edge_weights) == 1.0))

    # x chunks
    x = np.asarray(x, dtype=np.float32)
    x_pad = np.zeros((NCORES, PC, F_IN), dtype=np.float32)
    for c in range(NCORES):
        x_pad[c, :RC] = x[RC * c:RC * (c + 1)]

    return dict(
        T_total=T_total, t_wq=t_wq, tile_of=tile_of, call_meta=call_meta,
        cnorm_pt=cnorm_pt, dstrel_pt=dstrel_pt, idx_blocks=idx_blocks,
        s2_pt=s2_pt, dinv_pt=dinv_pt, ones_fast=ones_fast, x_pad=x_pad,
    )


def _build_program(meta, reps=1, skip_gather=False, skip_scatter=False, skip_collective=False, use_shared=True, ones_fast=False, debug_o1=False, dbgL=0):
    T_total = meta["T_total"]
    t_wq = meta["t_wq"]
    tile_of = meta["tile_of"]
    call_meta = meta["call_meta"]
    f32 = mybir.dt.float32

    nc = bacc.Bacc("TRN2", target_bir_lowering=False, debug=False,
                   num_devices=NCORES, num_swdge_queues=4)

    t_x = nc.dram_tensor("x_c", [PC * F_IN], f32, kind="ExternalInput").ap()
    t_idx = nc.dram_tensor("idxb", [T_total * 1024], mybir.dt.int16, kind="ExternalInput").ap()
    t_cnorm = nc.dram_tensor("cnorm", [128 * T_total], f32, kind="ExternalInput").ap()
    t_dstrel = nc.dram_tensor("dstrel", [128 * T_total], f32, kind="ExternalInput").ap()
    t_s2 = nc.dram_tensor("s2", [128 * NG], f32, kind="ExternalInput").ap()
    t_dinv = nc.dram_tensor("dinv", [128 * NG], f32, kind="ExternalInput").ap()
    t_w1 = nc.dram_tensor("w1", [F_IN, HID], f32, kind="ExternalInput").ap()
    t_w2 = nc.dram_tensor("w2", [HID, HID], f32, kind="ExternalInput").ap()
    t_w3 = nc.dram_tensor("w3", [HID, HID], f32, kind="ExternalInput").ap()
    t_wl = nc.dram_tensor("wl", [3 * HID, NCLS], f32, kind="ExternalInput").ap()
    t_brep = nc.dram_tensor("brep", [3, 128, HID], f32, kind="ExternalInput").ap()
    t_blrep = nc.dram_tensor("blrep", [128, NCLS], f32, kind="ExternalInput").ap()
    t_iota = nc.dram_tensor("iota", [128, W], f32, kind="ExternalInput").ap()
    t_eye = nc.dram_tensor("eye", [128, 128], f32, kind="ExternalInput").ap()
    t_y = nc.dram_tensor("y", [PC * NCLS], f32, kind="ExternalOutput").ap()

    with tile.TileContext(nc) as tc:
        with (
            tc.tile_pool(name="const", bufs=1) as cp,
            tc.tile_pool(name="resident", bufs=1) as rp,
            tc.tile_pool(name="work", bufs=3) as wp,
            tc.tile_pool(name="msgp", bufs=2) as mp,
            tc.tile_pool(name="sp", bufs=6) as spool,
            tc.tile_pool(name="psum", bufs=2, space="PSUM") as pp,
            tc.tile_pool(name="psum2", bufs=2, space="PSUM") as pp2,
            tc.tile_pool(name="dram", bufs=1, space="DRAM") as dp,
        ):
            # ---- constants / residents ----
            w1_t = cp.tile([F_IN, HID], f32); nc.sync.dma_start(out=w1_t[:], in_=t_w1[:])
            w2_t = cp.tile([HID, HID], f32); nc.sync.dma_start(out=w2_t[:], in_=t_w2[:])
            w3_t = cp.tile([HID, HID], f32); nc.sync.dma_start(out=w3_t[:], in_=t_w3[:])
            wl_ts = []
            for L in range(3):
                wt = cp.tile([HID, NCLS], f32, tag=f"wl{L}", name=f"wl{L}")
                nc.sync.dma_start(out=wt[:], in_=t_wl[HID * L:HID * (L + 1), :])
                wl_ts.append(wt)
            brep_t = cp.tile([128, 3, HID], f32)
            nc.sync.dma_start(out=brep_t[:], in_=t_brep.rearrange("l p h -> p l h"))
            blrep_t = cp.tile([128, NCLS], f32); nc.sync.dma_start(out=blrep_t[:], in_=t_blrep[:])
            iota_t = cp.tile([128, W], f32); nc.sync.dma_start(out=iota_t[:], in_=t_iota[:])
            eye_t = cp.tile([128, 128], f32); nc.sync.dma_start(out=eye_t[:], in_=t_eye[:])
            cnorm_t = rp.tile([128, T_total], f32)
            nc.sync.dma_start(out=cnorm_t[:], in_=t_cnorm.rearrange("(p t) -> p t", t=T_total))
            dstrel_t = rp.tile([128, T_total], f32)
            nc.sync.dma_start(out=dstrel_t[:], in_=t_dstrel.rearrange("(p t) -> p t", t=T_total))
            s2_t = rp.tile([128, NG], f32)
            nc.sync.dma_start(out=s2_t[:], in_=t_s2.rearrange("(p g) -> p g", g=NG))
            dinv_t = rp.tile([128, NG], f32)
            nc.sync.dma_start(out=dinv_t[:], in_=t_dinv.rearrange("(p g) -> p g", g=NG))

            h_own = rp.tile([128, NG, HID], f32)          # own chunk of current table (pre-pad)
            o_bufs = [rp.tile([128, NG, HID], f32, tag=f"o{L}", name=f"o{L}") for L in range(3)]

            in_bs = [dp.tile([PC * ELEM], f32, tag=f"inb{L}", name=f"inb{L}") for L in range(3)]

            for _rep in range(reps):
                # Shared tensors allow a single writer instruction; allocate
                # per rep so each rep's AllGather writes a fresh tensor.
                tables = [dp.tile([NP * ELEM], f32, tag=f"table{L}_r{_rep}",
                                  name=f"table{L}_r{_rep}",
                                  addr_space="Shared" if use_shared else "Local")
                          for L in range(3)]
                x_v = t_x.rearrange("(g p f) -> g p f", p=128, f=F_IN)

                def chain_write(L, g, o_ap):
                    """From o tile [128, HID] compute h = o @ W_{L+1}, write padded
                    row block to in_bs[L+1] and h_own."""
                    wn = [None, w2_t, w3_t][L + 1]
                    ot_ps = pp2.tile([HID, 128], f32, tag="tps")
                    nc.tensor.transpose(out=ot_ps[:], in_=o_ap, identity=eye_t[:])
                    ot_sb = wp.tile([HID, 128], f32, tag="otsb")
                    nc.vector.tensor_copy(out=ot_sb[:], in_=ot_ps[:])
                    h_ps = pp.tile([128, HID], f32, tag="hps")
                    nc.tensor.matmul(h_ps[:], lhsT=ot_sb[:], rhs=wn[:], start=True, stop=True)
                    h64 = wp.tile([128, ELEM], f32, tag="h64")
                    nc.vector.memset(h64[:, HID:], 0.0)
                    if ones_fast:
                        nc.vector.tensor_scalar(out=h64[:, :HID], in0=h_ps[:],
                                                scalar1=dinv_t[:, g:g + 1], scalar2=None,
                                                op0=mybir.AluOpType.mult)
                    else:
                        nc.vector.tensor_copy(out=h64[:, :HID], in_=h_ps[:])
                    nc.vector.tensor_copy(out=h_own[:, g, :], in_=h64[:, :HID])
                    nc.sync.dma_start(
                        out=in_bs[L + 1][:].rearrange("(g p e) -> g p e", p=128, e=ELEM)[g],
                        in_=h64[:])

                # ---- layer 1 table: h1 = x @ W1 ----
                for g in range(NG):
                    xt = wp.tile([128, F_IN], f32, tag="xt")
                    nc.sync.dma_start(out=xt[:], in_=x_v[g])
                    xT_ps = pp2.tile([128, 128], f32, tag="tps")
                    nc.tensor.transpose(out=xT_ps[:], in_=xt[:], identity=eye_t[:])
                    xT_sb = wp.tile([128, F_IN], f32, tag="xTsb")
                    nc.vector.tensor_copy(out=xT_sb[:], in_=xT_ps[:])
                    h_ps = pp.tile([128, HID], f32, tag="hps")
                    nc.tensor.matmul(h_ps[:], lhsT=xT_sb[:], rhs=w1_t[:], start=True, stop=True)
                    h64 = wp.tile([128, ELEM], f32, tag="h64")
                    nc.vector.memset(h64[:, HID:], 0.0)
                    if ones_fast:
                        nc.vector.tensor_scalar(out=h64[:, :HID], in0=h_ps[:],
                                                scalar1=dinv_t[:, g:g + 1], scalar2=None,
                                                op0=mybir.AluOpType.mult)
                    else:
                        nc.vector.tensor_copy(out=h64[:, :HID], in_=h_ps[:])
                    nc.vector.tensor_copy(out=h_own[:, g, :], in_=h64[:, :HID])
                    nc.sync.dma_start(
                        out=in_bs[0][:].rearrange("(g p e) -> g p e", p=128, e=ELEM)[g],
                        in_=h64[:])

                # ---- layers ----
                for L in range(3):
                    if not skip_collective:
                        nc.gpsimd.collective_compute(
                            "AllGather", mybir.AluOpType.bypass,
                            replica_groups=[list(range(NCORES))],
                            ins=[in_bs[L][:]], outs=[tables[L][:]])
                    else:
                        # timing stand-in: 8 local copies emulate a perfect
                        # shared-memory allgather (correctness broken)
                        for c in range(NCORES):
                            nc.sync.dma_start(
                                out=tables[L][PC * ELEM * c:PC * ELEM * (c + 1)],
                                in_=in_bs[L][:])
                    tab_q = [tables[L][:].rearrange("(n e) -> n e", e=ELEM)[Q4 * q:Q4 * (q + 1)]
                             for q in range(4)]

                    # supergroup tile extents
                    sg_t0 = [min(t0 for (s, q, t0, nt) in call_meta if s == sg)
                             for sg in range(NSG)]
                    sg_t1 = [max(t0 + nt for (s, q, t0, nt) in call_meta if s == sg)
                             for sg in range(NSG)]

                    for sg in range(NSG):
                        t0s, t1s = sg_t0[sg], sg_t1[sg]
                        nts = t1s - t0s
                        msg = mp.tile([128, nts, ELEM], f32, tag="msg")
                        if skip_gather:
                            nc.gpsimd.memset(msg[:], 0.0)
                        for (s, q, t0, nt) in call_meta:
                            if s != sg or nt == 0:
                                continue
                            nidx = nt * 128
                            idxt = wp.tile([128, nt * 8], mybir.dt.int16, tag="idxt")
                            nc.sync.dma_start(
                                out=idxt[:],
                                in_=t_idx[t0 * 1024:t0 * 1024 + nidx * 8]
                                    .rearrange("(p n) -> p n", p=128))
                            if not skip_gather:
                                nc.gpsimd.dma_gather(
                                    out_ap=msg[:, t0 - t0s:t0 - t0s + nt, :],
                                    in_ap=tab_q[q],
                                    idxs_ap=idxt[:],
                                    num_idxs=nidx, num_idxs_reg=nidx,
                                    elem_size=ELEM, elem_step=ELEM,
                                    single_packet=False, queue_num=q)
                            if not ones_fast:
                                nc.vector.tensor_tensor(
                                    out=msg[:, t0 - t0s:t0 - t0s + nt, :HID],
                                    in0=msg[:, t0 - t0s:t0 - t0s + nt, :HID],
                                    in1=cnorm_t[:, t0:t0 + nt].unsqueeze(-1).to_broadcast([128, nt, HID]),
                                    op=mybir.AluOpType.mult)

                        # two psum groups (2 windows each) per supergroup
                        for half in range(SGW // 2):
                            agg_ps = pp.tile([128, HID], f32, tag="agg")
                            if skip_scatter:
                                nc.vector.memset(agg_ps[:], 0.0)
                            g = sg * 2 + half  # node group index (128 dst)
                            for wi in range(2):
                                w_i = sg * SGW + half * 2 + wi
                                tiles_w = []
                                for q in range(4):
                                    for k in range(t_wq[w_i, q]):
                                        tiles_w.append(tile_of[(w_i, q, k)])
                                # S tiles in runs of up to 8 consecutive
                                runs = []
                                for t in tiles_w:
                                    if runs and runs[-1][0] + runs[-1][1] == t and runs[-1][1] < 8:
                                        runs[-1] = (runs[-1][0], runs[-1][1] + 1)
                                    else:
                                        runs.append((t, 1))
                                s_tiles = {}
                                for (rt, rn) in (runs if not skip_scatter else []):
                                    st = spool.tile([128, 8, W], f32, tag="S")
                                    nc.vector.tensor_tensor(
                                        out=st[:, :rn, :],
                                        in0=dstrel_t[:, rt:rt + rn].unsqueeze(-1).to_broadcast([128, rn, W]),
                                        in1=iota_t[:].unsqueeze(1).to_broadcast([128, rn, W]),
                                        op=mybir.AluOpType.is_equal)
                                    for j in range(rn):
                                        s_tiles[rt + j] = st[:, j, :]
                                for ti, t in (enumerate(tiles_w) if not skip_scatter else []):
                                    nc.tensor.matmul(
                                        agg_ps[64 * wi:64 * (wi + 1), :],
                                        lhsT=s_tiles[t],
                                        rhs=msg[:, t - t0s, :HID],
                                        start=(ti == 0), stop=(ti == len(tiles_w) - 1),
                                        skip_group_check=True)
                            # ---- postproc for node group g ----
                            A = wp.tile([128, HID], f32, tag="A")
                            if ones_fast:
                                # table rows are pre-scaled by dinv[src]; out =
                                # dinv[d]*(agg + h_own) + b
                                tmp = wp.tile([128, HID], f32, tag="tmp")
                                nc.vector.tensor_tensor(out=tmp[:], in0=agg_ps[:],
                                                        in1=h_own[:, g, :],
                                                        op=mybir.AluOpType.add)
                                nc.vector.tensor_scalar(
                                    out=tmp[:], in0=tmp[:], scalar1=dinv_t[:, g:g + 1],
                                    scalar2=None, op0=mybir.AluOpType.mult)
                                nc.vector.tensor_tensor(out=A[:], in0=tmp[:], in1=brep_t[:, L, :],
                                                        op=mybir.AluOpType.add)
                            else:
                                nc.vector.tensor_tensor(out=A[:], in0=agg_ps[:], in1=brep_t[:, L, :],
                                                        op=mybir.AluOpType.add)
                                tmp = wp.tile([128, HID], f32, tag="tmp")
                                nc.vector.tensor_scalar(
                                    out=tmp[:], in0=h_own[:, g, :], scalar1=s2_t[:, g:g + 1],
                                    scalar2=None, op0=mybir.AluOpType.mult)
                                nc.vector.tensor_tensor(out=A[:], in0=A[:], in1=tmp[:],
                                                        op=mybir.AluOpType.add)
                            sq = wp.tile([128, HID], f32, tag="sq")
                            n2 = wp.tile([128, 1], f32, tag="n2")
                            nc.scalar.activation(out=sq[:], in_=A[:],
                                                 func=mybir.ActivationFunctionType.Square,
                                                 accum_out=n2[:])
                            nc.vector.tensor_scalar(out=n2[:], in0=n2[:], scalar1=EPS2,
                                                    scalar2=None, op0=mybir.AluOpType.max)
                            rt_ = wp.tile([128, 1], f32, tag="rt")
                            nc.scalar.activation(out=rt_[:], in_=n2[:],
                                                 func=mybir.ActivationFunctionType.Sqrt)
                            rinv = wp.tile([128, 1], f32, tag="rinv")
                            nc.vector.reciprocal(out=rinv[:], in_=rt_[:])
                            nc.scalar.activation(out=o_bufs[L][:, g, :], in_=A[:],
                                                 func=mybir.ActivationFunctionType.Relu,
                                                 scale=rinv[:])
                            if L < 2:
                                chain_write(L, g, o_bufs[L][:, g, :])

                # ---- head: y = [o1|o2|o3] @ W_lin + b ----
                y_v = t_y.rearrange("(g p k) -> g p k", p=128, k=NCLS)
                for g in range(NG):
                    y_ps = pp.tile([128, NCLS], f32, tag="hps")
                    for L in range(3):
                        ot_ps = pp2.tile([HID, 128], f32, tag="tps")
                        nc.tensor.transpose(out=ot_ps[:], in_=o_bufs[L][:, g, :], identity=eye_t[:])
                        ot_sb = wp.tile([HID, 128], f32, tag="otsb")
                        nc.vector.tensor_copy(out=ot_sb[:], in_=ot_ps[:])
                        nc.tensor.matmul(y_ps[:], lhsT=ot_sb[:], rhs=wl_ts[L][:],
                                         start=(L == 0), stop=(L == 2), skip_group_check=True)
                    y_sb = wp.tile([128, NCLS], f32, tag="ysb")
                    if debug_o1:
                        nc.vector.tensor_copy(out=y_sb[:], in_=o_bufs[dbgL][:, g, :NCLS])
                    else:
                        nc.vector.tensor_tensor(out=y_sb[:], in0=y_ps[:], in1=blrep_t[:],
                                                op=mybir.AluOpType.add)
                    nc.sync.dma_start(out=y_v[g], in_=y_sb[:])

    nc.compile()
    return nc


def _make_in_maps(meta, W1, b1, W2, b2, W3, b3, W_lin, b_lin):
    brep = np.stack([np.tile(np.asarray(b, np.float32)[None, :], (128, 1))
                     for b in (b1, b2, b3)])           # [3,128,HID]
    blrep = np.tile(np.asarray(b_lin, np.float32)[None, :], (128, 1))
    iota = np.tile(np.arange(W, dtype=np.float32)[None, :], (128, 1))
    eye = np.eye(128, dtype=np.float32)
    maps = []
    for c in range(NCORES):
        maps.append({
            "x_c": meta["x_pad"][c].reshape(-1),
            "idxb": meta["idx_blocks"][c],
            "cnorm": meta["cnorm_pt"][c],
            "dstrel": meta["dstrel_pt"][c],
            "s2": meta["s2_pt"][c],
            "dinv": meta["dinv_pt"][c],
            "w1": np.asarray(W1, np.float32), "w2": np.asarray(W2, np.float32),
            "w3": np.asarray(W3, np.float32), "wl": np.asarray(W_lin, np.float32),
            "brep": brep, "blrep": blrep, "iota": iota, "eye": eye,
        })
    return maps


def kernel(x, edge_index, edge_weights, W1, b1, W2, b2, W3, b3, W_lin, b_lin):
    meta = _host_prep(x, edge_index, edge_weights)
    key = ("prog", meta["ones_fast"])
    if key not in _cache:
        _cache[key] = _build_program(meta, ones_fast=meta["ones_fast"])
    nc = _cache[key]
    in_maps = _make_in_maps(meta, W1, b1, W2, b2, W3, b3, W_lin, b_lin)
    res = run_bass_kernel_spmd(nc, in_maps, core_ids=list(range(NCORES)))
    ys = [res.results[c]["y"].reshape(PC, NCLS)[:RC] for c in range(NCORES)]
    return np.concatenate(ys, axis=0).astype(np.float32)



# revision 19
# speedup vs baseline: 1.1506x; 1.0141x over previous
"""3-layer GCN node predictor on 8 Trainium2 NeuronCores (Bass/Tile SPMD).

Strategy (graph/data parallel, per sharding hint):
- Nodes sharded into 8 contiguous chunks (12544 padded rows per core); each
  core aggregates the in-edges of its own dst nodes.
- Per layer, the gather table T_L = o_{L-1} @ W_L ([100352, 64] fp32, 256B
  rows) is built shard-wise and AllGathered to every core's DRAM.
- Per-edge gather of T_L[src] uses gpsimd dma_gather (int16 indices ->
  4 table quarters of 25088 rows), round-robin over 4 SWDGE queues.
- Scatter-add uses TensorE: one-hot S [128 edges, 64 dst] built on DVE via
  is_equal against an iota row, matmul S.T @ msg accumulated in PSUM.
- Self loops are applied node-wise from the SBUF-resident own chunk.
"""
import numpy as np

import concourse.bass as bass
import concourse.bacc as bacc
import concourse.tile as tile
import concourse.mybir as mybir
from concourse.bass_utils import run_bass_kernel_spmd

NCORES = 8
N = 100000
E = 3200000
F_IN = 128
HID = 32
NCLS = 10
RC = 12500          # real nodes per core
PC = 12544          # padded nodes per core (98 * 128)
NP = PC * NCORES    # padded total nodes (100352)
# Position-based quarters: sub-chunk boundaries within each core (multiples of
# 128). Quarter q holds rows [c*SUBS[q] + off] for off in [B[q], B[q+1]) of
# every core c, so an AllGather of the in_b slice [B[q], B[q+1]) over all 8
# cores materializes exactly quarter q. All quarter row counts < 32768 (int16).
QB = [0, 3200, 6400, 9472, 12544]
SUBS = [QB[i + 1] - QB[i] for i in range(4)]      # 3200, 3200, 3072, 3072
QROWS = [8 * s for s in SUBS]                     # 25600, 25600, 24576, 24576
ELEM = 64           # table row elements (256B rows)
W = 64              # dst window
NWIN = PC // W      # 196 windows per core
SGW = 4             # windows per supergroup
NSG = NWIN // SGW   # 49
NG = PC // 128      # 98 node groups of 128
EPS2 = 1e-24

_cache = {}


def _host_prep(x, edge_index, edge_weights):
    src = np.asarray(edge_index[0], dtype=np.int64)
    dst = np.asarray(edge_index[1], dtype=np.int64)
    ew = np.asarray(edge_weights, dtype=np.float64)

    deg = np.bincount(dst, weights=ew, minlength=N) + 1.0
    dinv = np.where(deg > 0, 1.0 / np.sqrt(deg), 0.0)
    cnorm_e = (dinv[src] * ew * dinv[dst]).astype(np.float32)
    s2 = (dinv * dinv).astype(np.float32)

    score = src // RC                             # owning core of src
    soff = src % RC                               # within-core offset
    sq = np.searchsorted(QB, soff, side="right") - 1          # quarter id
    qb = np.asarray(QB)[sq]
    ssub = np.asarray(SUBS)[sq]
    srow = score * ssub + (soff - qb)             # row within quarter

    per_core = []
    for c in range(NCORES):
        m = (dst >= RC * c) & (dst < RC * (c + 1))
        es = srow[m]
        ed = dst[m] - RC * c
        en = cnorm_e[m]
        w_id = ed // W
        q_id = sq[m]
        order = np.lexsort((es, q_id, w_id))      # sort by (w, q, src-row):
        # row-ascending descriptors within each (w, q) run give the HBM
        # sequential-ish locality; scatter is order-agnostic within a window.
        per_core.append((es[order], ed[order], en[order],
                         w_id[order], q_id[order]))

    # per (w, q) counts and max over cores
    counts = np.zeros((NCORES, NWIN, 4), dtype=np.int64)
    for c in range(NCORES):
        _, _, _, w_id, q_id = per_core[c]
        np.add.at(counts[c], (w_id, q_id), 1)
    cmax = counts.max(axis=0)
    t_wq = (cmax + 127) // 128                    # tiles per (w, q)
    for w_i in range(NWIN):
        if t_wq[w_i].sum() == 0:
            t_wq[w_i, 0] = 1

    # global tile order: (sg, q, w, k)
    tile_of = {}
    T_total = 0
    call_meta = []                                # (sg, q, t0, ntiles)
    for sg in range(NSG):
        for q in range(4):
            t0 = T_total
            for w_i in range(sg * SGW, (sg + 1) * SGW):
                for k in range(t_wq[w_i, q]):
                    tile_of[(w_i, q, k)] = T_total
                    T_total += 1
            call_meta.append((sg, q, t0, T_total - t0))

    # slot arrays
    idx16 = np.zeros((T_total * 128,), dtype=np.int16)
    cnorm = np.zeros((T_total * 128,), dtype=np.float32)
    dstrel = np.full((T_total * 128,), -1.0, dtype=np.float32)
    idx16_all = np.zeros((NCORES, T_total * 128), dtype=np.int16)
    cnorm_all = np.zeros((NCORES, T_total * 128), dtype=np.float32)
    dstrel_all = np.full((NCORES, T_total * 128), -1.0, dtype=np.float32)
    for c in range(NCORES):
        es, ed, en, w_id, q_id = per_core[c]
        # position within (w, q) run
        keys = w_id * 4 + q_id
        # edges already sorted by (w, q); rank within group:
        boundaries = np.flatnonzero(np.diff(keys, prepend=-1))
        ranks = np.arange(len(keys)) - np.repeat(boundaries, np.diff(np.append(boundaries, len(keys))))
        k_tile = ranks // 128
        k_part = ranks % 128
        gtile = np.array([tile_of[(w, q, k)] for (w, q, k) in zip(w_id, q_id, k_tile)])
        slot = gtile * 128 + k_part
        idx16_all[c, slot] = es.astype(np.int16)
        cnorm_all[c, slot] = en
        dstrel_all[c, slot] = (ed - w_id * W).astype(np.float32)
        # pad slots: repeat the run's last real row index (cheap row-buffer
        # hit next to the run's tail) instead of defaulting to row 0
        cnt_wq = np.bincount(keys, minlength=NWIN * 4).reshape(NWIN, 4)
        for w in range(NWIN):
            for q in range(4):
                nt = t_wq[w, q]
                cnt = cnt_wq[w, q]
                if nt == 0 or cnt == 0 or cnt == nt * 128:
                    continue
                t0 = tile_of[(w, q, 0)]
                idx16_all[c, t0 * 128 + cnt:(t0 + nt) * 128] = \
                    idx16_all[c, t0 * 128 + cnt - 1]

    # device layouts
    # cnorm/dstrel resident [128, T]: flat p * T + t; slot = t*128 + p
    def to_pt(a):
        return np.ascontiguousarray(a.reshape(-1, T_total, 128).transpose(0, 2, 1)).reshape(NCORES, -1)

    cnorm_pt = to_pt(cnorm_all)
    dstrel_pt = to_pt(dstrel_all)

    # idx per call: wrapped [128, 8*ntiles] int16, idx j of call at [j%16, j//16],
    # replicated across the 8 groups of 16 partitions. Flattened per call.
    idx_blocks = np.zeros((NCORES, T_total * 1024), dtype=np.int16)
    for (sg, q, t0, nt) in call_meta:
        if nt == 0:
            continue
        nidx = nt * 128
        for c in range(NCORES):
            blk = idx16_all[c, t0 * 128:(t0 + nt) * 128]
            wrp = blk.reshape(nidx // 16, 16).T              # [16, nidx/16]
            rep = np.tile(wrp, (8, 1))                       # [128, nidx/16]
            idx_blocks[c, t0 * 1024:t0 * 1024 + nidx * 8] = rep.reshape(-1)

    # s2/dinv resident [128, NG]: flat p * NG + g ; node g*128+p
    def node_pt(v):
        pad = np.zeros((NCORES, PC), dtype=np.float32)
        for c in range(NCORES):
            pad[c, :RC] = v[RC * c:RC * (c + 1)]
        return np.ascontiguousarray(pad.reshape(NCORES, NG, 128).transpose(0, 2, 1)).reshape(NCORES, -1)

    s2_pt = node_pt(s2)
    dinv_pt = node_pt(dinv.astype(np.float32))
    ones_fast = bool(np.all(np.asarray(edge_weights) == 1.0))

    # x chunks
    x = np.asarray(x, dtype=np.float32)
    x_pad = np.zeros((NCORES, PC, F_IN), dtype=np.float32)
    for c in range(NCORES):
        x_pad[c, :RC] = x[RC * c:RC * (c + 1)]

    return dict(
        T_total=T_total, t_wq=t_wq, tile_of=tile_of, call_meta=call_meta,
        cnorm_pt=cnorm_pt, dstrel_pt=dstrel_pt, idx_blocks=idx_blocks,
        s2_pt=s2_pt, dinv_pt=dinv_pt, ones_fast=ones_fast, x_pad=x_pad,
    )


def _build_program(meta, reps=1, skip_gather=False, skip_scatter=False, skip_collective=False, use_shared=True, ones_fast=False, debug_o1=False, dbgL=0):
    T_total = meta["T_total"]
    t_wq = meta["t_wq"]
    tile_of = meta["tile_of"]
    call_meta = meta["call_meta"]
    f32 = mybir.dt.float32

    nc = bacc.Bacc("TRN2", target_bir_lowering=False, debug=False,
                   num_devices=NCORES, num_swdge_queues=4)

    t_x = nc.dram_tensor("x_c", [PC * F_IN], f32, kind="ExternalInput").ap()
    t_idx = nc.dram_tensor("idxb", [T_total * 1024], mybir.dt.int16, kind="ExternalInput").ap()
    t_cnorm = nc.dram_tensor("cnorm", [128 * T_total], f32, kind="ExternalInput").ap()
    t_dstrel = nc.dram_tensor("dstrel", [128 * T_total], f32, kind="ExternalInput").ap()
    t_s2 = nc.dram_tensor("s2", [128 * NG], f32, kind="ExternalInput").ap()
    t_dinv = nc.dram_tensor("dinv", [128 * NG], f32, kind="ExternalInput").ap()
    t_w1 = nc.dram_tensor("w1", [F_IN, HID], f32, kind="ExternalInput").ap()
    t_w2 = nc.dram_tensor("w2", [HID, HID], f32, kind="ExternalInput").ap()
    t_w3 = nc.dram_tensor("w3", [HID, HID], f32, kind="ExternalInput").ap()
    t_wl = nc.dram_tensor("wl", [3 * HID, NCLS], f32, kind="ExternalInput").ap()
    t_brep = nc.dram_tensor("brep", [3, 128, HID], f32, kind="ExternalInput").ap()
    t_blrep = nc.dram_tensor("blrep", [128, NCLS], f32, kind="ExternalInput").ap()
    t_iota = nc.dram_tensor("iota", [128, W], f32, kind="ExternalInput").ap()
    t_eye = nc.dram_tensor("eye", [128, 128], f32, kind="ExternalInput").ap()
    t_y = nc.dram_tensor("y", [PC * NCLS], f32, kind="ExternalOutput").ap()

    with tile.TileContext(nc) as tc:
        with (
            tc.tile_pool(name="const", bufs=1) as cp,
            tc.tile_pool(name="resident", bufs=1) as rp,
            tc.tile_pool(name="work", bufs=3) as wp,
            tc.tile_pool(name="msgp", bufs=3) as mp,
            tc.tile_pool(name="sp", bufs=8) as spool,
            tc.tile_pool(name="psum", bufs=3, space="PSUM") as pp,
            tc.tile_pool(name="psum2", bufs=2, space="PSUM") as pp2,
            tc.tile_pool(name="dram", bufs=1, space="DRAM") as dp,
        ):
            # ---- constants / residents ----
            w1_t = cp.tile([F_IN, HID], f32); nc.sync.dma_start(out=w1_t[:], in_=t_w1[:])
            w2_t = cp.tile([HID, HID], f32); nc.sync.dma_start(out=w2_t[:], in_=t_w2[:])
            w3_t = cp.tile([HID, HID], f32); nc.sync.dma_start(out=w3_t[:], in_=t_w3[:])
            wl_ts = []
            for L in range(3):
                wt = cp.tile([HID, NCLS], f32, tag=f"wl{L}", name=f"wl{L}")
                nc.sync.dma_start(out=wt[:], in_=t_wl[HID * L:HID * (L + 1), :])
                wl_ts.append(wt)
            brep_t = cp.tile([128, 3, HID], f32)
            nc.sync.dma_start(out=brep_t[:], in_=t_brep.rearrange("l p h -> p l h"))
            blrep_t = cp.tile([128, NCLS], f32); nc.sync.dma_start(out=blrep_t[:], in_=t_blrep[:])
            iota_t = cp.tile([128, W], f32); nc.sync.dma_start(out=iota_t[:], in_=t_iota[:])
            eye_t = cp.tile([128, 128], f32); nc.sync.dma_start(out=eye_t[:], in_=t_eye[:])
            cnorm_t = rp.tile([128, T_total], f32)
            nc.sync.dma_start(out=cnorm_t[:], in_=t_cnorm.rearrange("(p t) -> p t", t=T_total))
            dstrel_t = rp.tile([128, T_total], f32)
            nc.sync.dma_start(out=dstrel_t[:], in_=t_dstrel.rearrange("(p t) -> p t", t=T_total))
            s2_t = rp.tile([128, NG], f32)
            nc.sync.dma_start(out=s2_t[:], in_=t_s2.rearrange("(p g) -> p g", g=NG))
            dinv_t = rp.tile([128, NG], f32)
            nc.sync.dma_start(out=dinv_t[:], in_=t_dinv.rearrange("(p g) -> p g", g=NG))

            h_own = rp.tile([128, NG, HID], f32)          # own chunk of current table (pre-pad)
            o_bufs = [rp.tile([128, NG, HID], f32, tag=f"o{L}", name=f"o{L}") for L in range(3)]

            in_bs = [dp.tile([PC * ELEM], f32, tag=f"inb{L}", name=f"inb{L}") for L in range(3)]

            for _rep in range(reps):
                # Shared tensors allow a single writer instruction; allocate
                # per rep so each rep's AllGather writes a fresh tensor.
                # One tensor per (layer, quarter): the 4 chunked AllGathers per
                # layer start as soon as their in_b sub-chunk is complete and
                # overlap with gathers against earlier quarters.
                qts = [[dp.tile([QROWS[q] * ELEM], f32, tag=f"qt{L}_{q}_r{_rep}",
                                name=f"qt{L}_{q}_r{_rep}",
                                addr_space="Shared" if use_shared else "Local")
                        for q in range(4)] for L in range(3)]
                x_v = t_x.rearrange("(g p f) -> g p f", p=128, f=F_IN)

                def chain_write(L, g, o_ap):
                    """From o tile [128, HID] compute h = o @ W_{L+1}, write padded
                    row block to in_bs[L+1] and h_own."""
                    wn = [None, w2_t, w3_t][L + 1]
                    ot_ps = pp2.tile([HID, 128], f32, tag="tps")
                    nc.tensor.transpose(out=ot_ps[:], in_=o_ap, identity=eye_t[:])
                    ot_sb = wp.tile([HID, 128], f32, tag="otsb")
                    nc.vector.tensor_copy(out=ot_sb[:], in_=ot_ps[:])
                    h_ps = pp.tile([128, HID], f32, tag="hps")
                    nc.tensor.matmul(h_ps[:], lhsT=ot_sb[:], rhs=wn[:], start=True, stop=True)
                    h64 = wp.tile([128, ELEM], f32, tag="h64")
                    nc.vector.memset(h64[:, HID:], 0.0)
                    if ones_fast:
                        nc.vector.tensor_scalar(out=h64[:, :HID], in0=h_ps[:],
                                                scalar1=dinv_t[:, g:g + 1], scalar2=None,
                                                op0=mybir.AluOpType.mult)
                    else:
                        nc.vector.tensor_copy(out=h64[:, :HID], in_=h_ps[:])
                    nc.vector.tensor_copy(out=h_own[:, g, :], in_=h64[:, :HID])
                    nc.sync.dma_start(
                        out=in_bs[L + 1][:].rearrange("(g p e) -> g p e", p=128, e=ELEM)[g],
                        in_=h64[:])

                # ---- layer 1 table: h1 = x @ W1 ----
                for g in range(NG):
                    xt = wp.tile([128, F_IN], f32, tag="xt")
                    nc.sync.dma_start(out=xt[:], in_=x_v[g])
                    xT_ps = pp2.tile([128, 128], f32, tag="tps")
                    nc.tensor.transpose(out=xT_ps[:], in_=xt[:], identity=eye_t[:])
                    xT_sb = wp.tile([128, F_IN], f32, tag="xTsb")
                    nc.vector.tensor_copy(out=xT_sb[:], in_=xT_ps[:])
                    h_ps = pp.tile([128, HID], f32, tag="hps")
                    nc.tensor.matmul(h_ps[:], lhsT=xT_sb[:], rhs=w1_t[:], start=True, stop=True)
                    h64 = wp.tile([128, ELEM], f32, tag="h64")
                    nc.vector.memset(h64[:, HID:], 0.0)
                    if ones_fast:
                        nc.vector.tensor_scalar(out=h64[:, :HID], in0=h_ps[:],
                                                scalar1=dinv_t[:, g:g + 1], scalar2=None,
                                                op0=mybir.AluOpType.mult)
                    else:
                        nc.vector.tensor_copy(out=h64[:, :HID], in_=h_ps[:])
                    nc.vector.tensor_copy(out=h_own[:, g, :], in_=h64[:, :HID])
                    nc.sync.dma_start(
                        out=in_bs[0][:].rearrange("(g p e) -> g p e", p=128, e=ELEM)[g],
                        in_=h64[:])

                def do_postproc(L, g, agg_ps):
                    A = wp.tile([128, HID], f32, tag="A")
                    if ones_fast:
                        # table rows are pre-scaled by dinv[src]; out =
                        # dinv[d]*(agg + h_own) + b
                        tmp = wp.tile([128, HID], f32, tag="tmp")
                        nc.vector.tensor_tensor(out=tmp[:], in0=agg_ps[:],
                                                in1=h_own[:, g, :],
                                                op=mybir.AluOpType.add)
                        nc.vector.tensor_scalar(
                            out=tmp[:], in0=tmp[:], scalar1=dinv_t[:, g:g + 1],
                            scalar2=None, op0=mybir.AluOpType.mult)
                        nc.vector.tensor_tensor(out=A[:], in0=tmp[:], in1=brep_t[:, L, :],
                                                op=mybir.AluOpType.add)
                    else:
                        nc.vector.tensor_tensor(out=A[:], in0=agg_ps[:], in1=brep_t[:, L, :],
                                                op=mybir.AluOpType.add)
                        tmp = wp.tile([128, HID], f32, tag="tmp")
                        nc.vector.tensor_scalar(
                            out=tmp[:], in0=h_own[:, g, :], scalar1=s2_t[:, g:g + 1],
                            scalar2=None, op0=mybir.AluOpType.mult)
                        nc.vector.tensor_tensor(out=A[:], in0=A[:], in1=tmp[:],
                                                op=mybir.AluOpType.add)
                    sq = wp.tile([128, HID], f32, tag="sq")
                    n2 = wp.tile([128, 1], f32, tag="n2")
                    nc.scalar.activation(out=sq[:], in_=A[:],
                                         func=mybir.ActivationFunctionType.Square,
                                         accum_out=n2[:])
                    nc.vector.tensor_scalar(out=n2[:], in0=n2[:], scalar1=EPS2,
                                            scalar2=None, op0=mybir.AluOpType.max)
                    rt_ = wp.tile([128, 1], f32, tag="rt")
                    nc.scalar.activation(out=rt_[:], in_=n2[:],
                                         func=mybir.ActivationFunctionType.Sqrt)
                    rinv = wp.tile([128, 1], f32, tag="rinv")
                    nc.vector.reciprocal(out=rinv[:], in_=rt_[:])
                    nc.scalar.activation(out=o_bufs[L][:, g, :], in_=A[:],
                                         func=mybir.ActivationFunctionType.Relu,
                                         scale=rinv[:])
                    if L < 2:
                        chain_write(L, g, o_bufs[L][:, g, :])

                pending = []

                # ---- layers ----
                for L in range(3):
                    for q in range(4):
                        in_slice = in_bs[L][QB[q] * ELEM:QB[q + 1] * ELEM]
                        if not skip_collective:
                            nc.gpsimd.collective_compute(
                                "AllGather", mybir.AluOpType.bypass,
                                replica_groups=[list(range(NCORES))],
                                ins=[in_slice], outs=[qts[L][q][:]])
                        else:
                            # timing stand-in: 8 local copies emulate a perfect
                            # shared-memory allgather (correctness broken)
                            for c in range(NCORES):
                                nc.sync.dma_start(
                                    out=qts[L][q][SUBS[q] * ELEM * c:SUBS[q] * ELEM * (c + 1)],
                                    in_=in_slice)
                    tab_q = [qts[L][q][:].rearrange("(n e) -> n e", e=ELEM)
                             for q in range(4)]

                    # supergroup tile extents
                    sg_t0 = [min(t0 for (s, q, t0, nt) in call_meta if s == sg)
                             for sg in range(NSG)]
                    sg_t1 = [max(t0 + nt for (s, q, t0, nt) in call_meta if s == sg)
                             for sg in range(NSG)]

                    for sg in range(NSG):
                        t0s, t1s = sg_t0[sg], sg_t1[sg]
                        nts = t1s - t0s
                        msg = mp.tile([128, nts, ELEM], f32, tag="msg")
                        if skip_gather:
                            nc.gpsimd.memset(msg[:], 0.0)
                        for (s, q, t0, nt) in call_meta:
                            if s != sg or nt == 0:
                                continue
                            nidx = nt * 128
                            idxt = wp.tile([128, nt * 8], mybir.dt.int16, tag="idxt")
                            nc.sync.dma_start(
                                out=idxt[:],
                                in_=t_idx[t0 * 1024:t0 * 1024 + nidx * 8]
                                    .rearrange("(p n) -> p n", p=128))
                            if not skip_gather:
                                nc.gpsimd.dma_gather(
                                    out_ap=msg[:, t0 - t0s:t0 - t0s + nt, :],
                                    in_ap=tab_q[q],
                                    idxs_ap=idxt[:],
                                    num_idxs=nidx, num_idxs_reg=nidx,
                                    elem_size=ELEM, elem_step=ELEM,
                                    single_packet=False, queue_num=q)
                            if not ones_fast:
                                nc.vector.tensor_tensor(
                                    out=msg[:, t0 - t0s:t0 - t0s + nt, :HID],
                                    in0=msg[:, t0 - t0s:t0 - t0s + nt, :HID],
                                    in1=cnorm_t[:, t0:t0 + nt].unsqueeze(-1).to_broadcast([128, nt, HID]),
                                    op=mybir.AluOpType.mult)

                        # two psum groups (2 windows each) per supergroup
                        for half in range(SGW // 2):
                            agg_ps = pp.tile([128, HID], f32, tag="agg")
                            if skip_scatter:
                                nc.vector.memset(agg_ps[:], 0.0)
                            g = sg * 2 + half  # node group index (128 dst)
                            for wi in range(2):
                                w_i = sg * SGW + half * 2 + wi
                                tiles_w = []
                                for q in range(4):
                                    for k in range(t_wq[w_i, q]):
                                        tiles_w.append(tile_of[(w_i, q, k)])
                                # S tiles in runs of up to 8 consecutive
                                runs = []
                                for t in tiles_w:
                                    if runs and runs[-1][0] + runs[-1][1] == t and runs[-1][1] < 8:
                                        runs[-1] = (runs[-1][0], runs[-1][1] + 1)
                                    else:
                                        runs.append((t, 1))
                                s_tiles = {}
                                for (rt, rn) in (runs if not skip_scatter else []):
                                    st = spool.tile([128, 8, W], f32, tag="S")
                                    nc.vector.tensor_tensor(
                                        out=st[:, :rn, :],
                                        in0=dstrel_t[:, rt:rt + rn].unsqueeze(-1).to_broadcast([128, rn, W]),
                                        in1=iota_t[:].unsqueeze(1).to_broadcast([128, rn, W]),
                                        op=mybir.AluOpType.is_equal)
                                    for j in range(rn):
                                        s_tiles[rt + j] = st[:, j, :]
                                for ti, t in (enumerate(tiles_w) if not skip_scatter else []):
                                    nc.tensor.matmul(
                                        agg_ps[64 * wi:64 * (wi + 1), :],
                                        lhsT=s_tiles[t],
                                        rhs=msg[:, t - t0s, :HID],
                                        start=(ti == 0), stop=(ti == len(tiles_w) - 1),
                                        skip_group_check=True)
                            pending.append((g, agg_ps))
                            if len(pending) > 2:
                                do_postproc(L, *pending.pop(0))
                    for item in pending:
                        do_postproc(L, *item)
                    pending.clear()

                # ---- head: y = [o1|o2|o3] @ W_lin + b ----
                y_v = t_y.rearrange("(g p k) -> g p k", p=128, k=NCLS)
                for g in range(NG):
                    y_ps = pp.tile([128, NCLS], f32, tag="hps")
                    for L in range(3):
                        ot_ps = pp2.tile([HID, 128], f32, tag="tps")
                        nc.tensor.transpose(out=ot_ps[:], in_=o_bufs[L][:, g, :], identity=eye_t[:])
                        ot_sb = wp.tile([HID, 128], f32, tag="otsb")
                        nc.vector.tensor_copy(out=ot_sb[:], in_=ot_ps[:])
                        nc.tensor.matmul(y_ps[:], lhsT=ot_sb[:], rhs=wl_ts[L][:],
                                         start=(L == 0), stop=(L == 2), skip_group_check=True)
                    y_sb = wp.tile([128, NCLS], f32, tag="ysb")
                    if debug_o1:
                        nc.vector.tensor_copy(out=y_sb[:], in_=o_bufs[dbgL][:, g, :NCLS])
                    else:
                        nc.vector.tensor_tensor(out=y_sb[:], in0=y_ps[:], in1=blrep_t[:],
                                                op=mybir.AluOpType.add)
                    nc.sync.dma_start(out=y_v[g], in_=y_sb[:])

    nc.compile()
    return nc


def _make_in_maps(meta, W1, b1, W2, b2, W3, b3, W_lin, b_lin):
    brep = np.stack([np.tile(np.asarray(b, np.float32)[None, :], (128, 1))
                     for b in (b1, b2, b3)])           # [3,128,HID]
    blrep = np.tile(np.asarray(b_lin, np.float32)[None, :], (128, 1))
    iota = np.tile(np.arange(W, dtype=np.float32)[None, :], (128, 1))
    eye = np.eye(128, dtype=np.float32)
    maps = []
    for c in range(NCORES):
        maps.append({
            "x_c": meta["x_pad"][c].reshape(-1),
            "idxb": meta["idx_blocks"][c],
            "cnorm": meta["cnorm_pt"][c],
            "dstrel": meta["dstrel_pt"][c],
            "s2": meta["s2_pt"][c],
            "dinv": meta["dinv_pt"][c],
            "w1": np.asarray(W1, np.float32), "w2": np.asarray(W2, np.float32),
            "w3": np.asarray(W3, np.float32), "wl": np.asarray(W_lin, np.float32),
            "brep": brep, "blrep": blrep, "iota": iota, "eye": eye,
        })
    return maps


def kernel(x, edge_index, edge_weights, W1, b1, W2, b2, W3, b3, W_lin, b_lin):
    meta = _host_prep(x, edge_index, edge_weights)
    key = ("prog", meta["ones_fast"])
    if key not in _cache:
        _cache[key] = _build_program(meta, ones_fast=meta["ones_fast"])
    nc = _cache[key]
    in_maps = _make_in_maps(meta, W1, b1, W2, b2, W3, b3, W_lin, b_lin)
    res = run_bass_kernel_spmd(nc, in_maps, core_ids=list(range(NCORES)))
    ys = [res.results[c]["y"].reshape(PC, NCLS)[:RC] for c in range(NCORES)]
    return np.concatenate(ys, axis=0).astype(np.float32)



# revision 23
# speedup vs baseline: 1.2010x; 1.0439x over previous
"""3-layer GCN node predictor on 8 Trainium2 NeuronCores (Bass/Tile SPMD).

Strategy (graph/data parallel, per sharding hint):
- Nodes sharded into 8 contiguous chunks (12544 padded rows per core); each
  core aggregates the in-edges of its own dst nodes.
- Per layer, the gather table T_L = o_{L-1} @ W_L ([100352, 64] fp32, 256B
  rows) is built shard-wise and AllGathered to every core's DRAM.
- Per-edge gather of T_L[src] uses gpsimd dma_gather (int16 indices ->
  4 table quarters of 25088 rows), round-robin over 4 SWDGE queues.
- Scatter-add uses TensorE: one-hot S [128 edges, 64 dst] built on DVE via
  is_equal against an iota row, matmul S.T @ msg accumulated in PSUM.
- Self loops are applied node-wise from the SBUF-resident own chunk.
"""
import numpy as np

import concourse.bass as bass
import concourse.bacc as bacc
import concourse.tile as tile
import concourse.mybir as mybir
from concourse.bass_utils import run_bass_kernel_spmd

NCORES = 8
N = 100000
E = 3200000
F_IN = 128
HID = 32
NCLS = 10
RC = 12500          # real nodes per core
PC = 12544          # padded nodes per core (98 * 128)
NP = PC * NCORES    # padded total nodes (100352)
# Position-based quarters: sub-chunk boundaries within each core (multiples of
# 128). Quarter q holds rows [c*SUBS[q] + off] for off in [B[q], B[q+1]) of
# every core c, so an AllGather of the in_b slice [B[q], B[q+1]) over all 8
# cores materializes exactly quarter q. All quarter row counts < 32768 (int16).
QB = [0, 3200, 6400, 9472, 12544]
SUBS = [QB[i + 1] - QB[i] for i in range(4)]      # 3200, 3200, 3072, 3072
QROWS = [8 * s for s in SUBS]                     # 25600, 25600, 24576, 24576
ELEM = 64           # table row elements (256B rows)
W = 128             # dst window (= node group)
NWIN = PC // W      # 98 windows per core
SGW = 2             # windows per supergroup
NSG = NWIN // SGW   # 49
NG = PC // 128      # 98 node groups of 128
EPS2 = 1e-24

_cache = {}


def _host_prep(x, edge_index, edge_weights):
    src = np.asarray(edge_index[0], dtype=np.int64)
    dst = np.asarray(edge_index[1], dtype=np.int64)
    ew = np.asarray(edge_weights, dtype=np.float64)

    deg = np.bincount(dst, weights=ew, minlength=N) + 1.0
    dinv = np.where(deg > 0, 1.0 / np.sqrt(deg), 0.0)
    cnorm_e = (dinv[src] * ew * dinv[dst]).astype(np.float32)
    s2 = (dinv * dinv).astype(np.float32)

    score = src // RC                             # owning core of src
    soff = src % RC                               # within-core offset
    sq = np.searchsorted(QB, soff, side="right") - 1          # quarter id
    qb = np.asarray(QB)[sq]
    ssub = np.asarray(SUBS)[sq]
    srow = score * ssub + (soff - qb)             # row within quarter

    per_core = []
    for c in range(NCORES):
        m = (dst >= RC * c) & (dst < RC * (c + 1))
        es = srow[m]
        ed = dst[m] - RC * c
        en = cnorm_e[m]
        w_id = ed // W
        q_id = sq[m]
        order = np.lexsort((es, q_id, w_id))      # sort by (w, q, src-row):
        # row-ascending descriptors within each (w, q) run give the HBM
        # sequential-ish locality; scatter is order-agnostic within a window.
        per_core.append((es[order], ed[order], en[order],
                         w_id[order], q_id[order]))

    # per (w, q) counts and max over cores
    counts = np.zeros((NCORES, NWIN, 4), dtype=np.int64)
    for c in range(NCORES):
        _, _, _, w_id, q_id = per_core[c]
        np.add.at(counts[c], (w_id, q_id), 1)
    cmax = counts.max(axis=0)
    t_wq = (cmax + 127) // 128                    # tiles per (w, q)
    for w_i in range(NWIN):
        if t_wq[w_i].sum() == 0:
            t_wq[w_i, 0] = 1

    # global tile order: (sg, q, w, k)
    tile_of = {}
    T_total = 0
    call_meta = []                                # (sg, q, t0, ntiles)
    for sg in range(NSG):
        for q in range(4):
            t0 = T_total
            for w_i in range(sg * SGW, (sg + 1) * SGW):
                for k in range(t_wq[w_i, q]):
                    tile_of[(w_i, q, k)] = T_total
                    T_total += 1
            call_meta.append((sg, q, t0, T_total - t0))

    # slot arrays
    idx16 = np.zeros((T_total * 128,), dtype=np.int16)
    cnorm = np.zeros((T_total * 128,), dtype=np.float32)
    dstrel = np.full((T_total * 128,), -1.0, dtype=np.float32)
    idx16_all = np.zeros((NCORES, T_total * 128), dtype=np.int16)
    cnorm_all = np.zeros((NCORES, T_total * 128), dtype=np.float32)
    dstrel_all = np.full((NCORES, T_total * 128), -1.0, dtype=np.float32)
    for c in range(NCORES):
        es, ed, en, w_id, q_id = per_core[c]
        # position within (w, q) run
        keys = w_id * 4 + q_id
        # edges already sorted by (w, q); rank within group:
        boundaries = np.flatnonzero(np.diff(keys, prepend=-1))
        ranks = np.arange(len(keys)) - np.repeat(boundaries, np.diff(np.append(boundaries, len(keys))))
        k_tile = ranks // 128
        k_part = ranks % 128
        gtile = np.array([tile_of[(w, q, k)] for (w, q, k) in zip(w_id, q_id, k_tile)])
        slot = gtile * 128 + k_part
        idx16_all[c, slot] = es.astype(np.int16)
        cnorm_all[c, slot] = en
        dstrel_all[c, slot] = (ed - w_id * W).astype(np.float32)
        # pad slots: repeat the run's last real row index (cheap row-buffer
        # hit next to the run's tail) instead of defaulting to row 0
        cnt_wq = np.bincount(keys, minlength=NWIN * 4).reshape(NWIN, 4)
        for w in range(NWIN):
            for q in range(4):
                nt = t_wq[w, q]
                cnt = cnt_wq[w, q]
                if nt == 0 or cnt == 0 or cnt == nt * 128:
                    continue
                t0 = tile_of[(w, q, 0)]
                idx16_all[c, t0 * 128 + cnt:(t0 + nt) * 128] = \
                    idx16_all[c, t0 * 128 + cnt - 1]

    # device layouts
    # cnorm/dstrel resident [128, T]: flat p * T + t; slot = t*128 + p
    def to_pt(a):
        return np.ascontiguousarray(a.reshape(-1, T_total, 128).transpose(0, 2, 1)).reshape(NCORES, -1)

    cnorm_pt = to_pt(cnorm_all)
    dstrel_pt = to_pt(dstrel_all)

    # idx per call: wrapped [128, 8*ntiles] int16, idx j of call at [j%16, j//16],
    # replicated across the 8 groups of 16 partitions. Flattened per call.
    idx_blocks = np.zeros((NCORES, T_total * 1024), dtype=np.int16)
    for (sg, q, t0, nt) in call_meta:
        if nt == 0:
            continue
        nidx = nt * 128
        for c in range(NCORES):
            blk = idx16_all[c, t0 * 128:(t0 + nt) * 128]
            wrp = blk.reshape(nidx // 16, 16).T              # [16, nidx/16]
            rep = np.tile(wrp, (8, 1))                       # [128, nidx/16]
            idx_blocks[c, t0 * 1024:t0 * 1024 + nidx * 8] = rep.reshape(-1)

    # s2/dinv resident [128, NG]: flat p * NG + g ; node g*128+p
    def node_pt(v):
        pad = np.zeros((NCORES, PC), dtype=np.float32)
        for c in range(NCORES):
            pad[c, :RC] = v[RC * c:RC * (c + 1)]
        return np.ascontiguousarray(pad.reshape(NCORES, NG, 128).transpose(0, 2, 1)).reshape(NCORES, -1)

    s2_pt = node_pt(s2)
    dinv_pt = node_pt(dinv.astype(np.float32))
    ones_fast = bool(np.all(np.asarray(edge_weights) == 1.0))

    # x chunks
    x = np.asarray(x, dtype=np.float32)
    x_pad = np.zeros((NCORES, PC, F_IN), dtype=np.float32)
    for c in range(NCORES):
        x_pad[c, :RC] = x[RC * c:RC * (c + 1)]

    return dict(
        T_total=T_total, t_wq=t_wq, tile_of=tile_of, call_meta=call_meta,
        cnorm_pt=cnorm_pt, dstrel_pt=dstrel_pt, idx_blocks=idx_blocks,
        s2_pt=s2_pt, dinv_pt=dinv_pt, ones_fast=ones_fast, x_pad=x_pad,
    )


def _build_program(meta, reps=1, skip_gather=False, skip_scatter=False, skip_collective=False, use_shared=True, ones_fast=False, debug_o1=False, dbgL=0):
    T_total = meta["T_total"]
    t_wq = meta["t_wq"]
    tile_of = meta["tile_of"]
    call_meta = meta["call_meta"]
    f32 = mybir.dt.float32

    nc = bacc.Bacc("TRN2", target_bir_lowering=False, debug=False,
                   num_devices=NCORES, num_swdge_queues=4)

    t_x = nc.dram_tensor("x_c", [PC * F_IN], f32, kind="ExternalInput").ap()
    t_idx = nc.dram_tensor("idxb", [T_total * 1024], mybir.dt.int16, kind="ExternalInput").ap()
    t_cnorm = nc.dram_tensor("cnorm", [128 * T_total], f32, kind="ExternalInput").ap()
    t_dstrel = nc.dram_tensor("dstrel", [128 * T_total], f32, kind="ExternalInput").ap()
    t_s2 = nc.dram_tensor("s2", [128 * NG], f32, kind="ExternalInput").ap()
    t_dinv = nc.dram_tensor("dinv", [128 * NG], f32, kind="ExternalInput").ap()
    t_w1 = nc.dram_tensor("w1", [F_IN, HID], f32, kind="ExternalInput").ap()
    t_w2 = nc.dram_tensor("w2", [HID, HID], f32, kind="ExternalInput").ap()
    t_w3 = nc.dram_tensor("w3", [HID, HID], f32, kind="ExternalInput").ap()
    t_wl = nc.dram_tensor("wl", [3 * HID, NCLS], f32, kind="ExternalInput").ap()
    t_brep = nc.dram_tensor("brep", [3, 128, HID], f32, kind="ExternalInput").ap()
    t_blrep = nc.dram_tensor("blrep", [128, NCLS], f32, kind="ExternalInput").ap()
    t_iota = nc.dram_tensor("iota", [128, W], f32, kind="ExternalInput").ap()
    t_eye = nc.dram_tensor("eye", [128, 128], f32, kind="ExternalInput").ap()
    t_y = nc.dram_tensor("y", [PC * NCLS], f32, kind="ExternalOutput").ap()

    with tile.TileContext(nc) as tc:
        with (
            tc.tile_pool(name="const", bufs=1) as cp,
            tc.tile_pool(name="resident", bufs=1) as rp,
            tc.tile_pool(name="work", bufs=3) as wp,
            tc.tile_pool(name="msgp", bufs=3) as mp,
            tc.tile_pool(name="sp", bufs=8) as spool,
            tc.tile_pool(name="psum", bufs=3, space="PSUM") as pp,
            tc.tile_pool(name="psum2", bufs=2, space="PSUM") as pp2,
            tc.tile_pool(name="dram", bufs=1, space="DRAM") as dp,
        ):
            # ---- constants / residents ----
            w1_t = cp.tile([F_IN, HID], f32); nc.sync.dma_start(out=w1_t[:], in_=t_w1[:])
            w2_t = cp.tile([HID, HID], f32); nc.sync.dma_start(out=w2_t[:], in_=t_w2[:])
            w3_t = cp.tile([HID, HID], f32); nc.sync.dma_start(out=w3_t[:], in_=t_w3[:])
            wl_ts = []
            for L in range(3):
                wt = cp.tile([HID, NCLS], f32, tag=f"wl{L}", name=f"wl{L}")
                nc.sync.dma_start(out=wt[:], in_=t_wl[HID * L:HID * (L + 1), :])
                wl_ts.append(wt)
            brep_t = cp.tile([128, 3, HID], f32)
            nc.sync.dma_start(out=brep_t[:], in_=t_brep.rearrange("l p h -> p l h"))
            blrep_t = cp.tile([128, NCLS], f32); nc.sync.dma_start(out=blrep_t[:], in_=t_blrep[:])
            iota_t = cp.tile([128, W], f32); nc.sync.dma_start(out=iota_t[:], in_=t_iota[:])
            eye_t = cp.tile([128, 128], f32); nc.sync.dma_start(out=eye_t[:], in_=t_eye[:])
            cnorm_t = rp.tile([128, T_total], f32)
            nc.sync.dma_start(out=cnorm_t[:], in_=t_cnorm.rearrange("(p t) -> p t", t=T_total))
            dstrel_t = rp.tile([128, T_total], f32)
            nc.sync.dma_start(out=dstrel_t[:], in_=t_dstrel.rearrange("(p t) -> p t", t=T_total))
            s2_t = rp.tile([128, NG], f32)
            nc.sync.dma_start(out=s2_t[:], in_=t_s2.rearrange("(p g) -> p g", g=NG))
            dinv_t = rp.tile([128, NG], f32)
            nc.sync.dma_start(out=dinv_t[:], in_=t_dinv.rearrange("(p g) -> p g", g=NG))

            h_own = rp.tile([128, NG, HID], f32)          # own chunk of current table (pre-pad)
            o_bufs = [rp.tile([128, NG, HID], f32, tag=f"o{L}", name=f"o{L}") for L in range(3)]

            in_bs = [dp.tile([PC * ELEM], f32, tag=f"inb{L}", name=f"inb{L}") for L in range(3)]

            for _rep in range(reps):
                # Shared tensors allow a single writer instruction; allocate
                # per rep so each rep's AllGather writes a fresh tensor.
                # One tensor per (layer, quarter): the 4 chunked AllGathers per
                # layer start as soon as their in_b sub-chunk is complete and
                # overlap with gathers against earlier quarters.
                qts = [[dp.tile([QROWS[q] * ELEM], f32, tag=f"qt{L}_{q}_r{_rep}",
                                name=f"qt{L}_{q}_r{_rep}",
                                addr_space="Shared" if use_shared else "Local")
                        for q in range(4)] for L in range(3)]
                x_v = t_x.rearrange("(g p f) -> g p f", p=128, f=F_IN)

                def chain_write(L, g, o_ap):
                    """From o tile [128, HID] compute h = o @ W_{L+1}, write padded
                    row block to in_bs[L+1] and h_own."""
                    wn = [None, w2_t, w3_t][L + 1]
                    ot_ps = pp2.tile([HID, 128], f32, tag="tps")
                    nc.tensor.transpose(out=ot_ps[:], in_=o_ap, identity=eye_t[:])
                    ot_sb = wp.tile([HID, 128], f32, tag="otsb")
                    nc.vector.tensor_copy(out=ot_sb[:], in_=ot_ps[:])
                    h_ps = pp.tile([128, HID], f32, tag="hps")
                    nc.tensor.matmul(h_ps[:], lhsT=ot_sb[:], rhs=wn[:], start=True, stop=True)
                    h64 = wp.tile([128, ELEM], f32, tag="h64")
                    nc.vector.memset(h64[:, HID:], 0.0)
                    if ones_fast:
                        nc.vector.tensor_scalar(out=h64[:, :HID], in0=h_ps[:],
                                                scalar1=dinv_t[:, g:g + 1], scalar2=None,
                                                op0=mybir.AluOpType.mult)
                    else:
                        nc.vector.tensor_copy(out=h64[:, :HID], in_=h_ps[:])
                    nc.vector.tensor_copy(out=h_own[:, g, :], in_=h64[:, :HID])
                    nc.sync.dma_start(
                        out=in_bs[L + 1][:].rearrange("(g p e) -> g p e", p=128, e=ELEM)[g],
                        in_=h64[:])

                # ---- layer 1 table: h1 = x @ W1 ----
                for g in range(NG):
                    xt = wp.tile([128, F_IN], f32, tag="xt")
                    nc.sync.dma_start(out=xt[:], in_=x_v[g])
                    xT_ps = pp2.tile([128, 128], f32, tag="tps")
                    nc.tensor.transpose(out=xT_ps[:], in_=xt[:], identity=eye_t[:])
                    xT_sb = wp.tile([128, F_IN], f32, tag="xTsb")
                    nc.vector.tensor_copy(out=xT_sb[:], in_=xT_ps[:])
                    h_ps = pp.tile([128, HID], f32, tag="hps")
                    nc.tensor.matmul(h_ps[:], lhsT=xT_sb[:], rhs=w1_t[:], start=True, stop=True)
                    h64 = wp.tile([128, ELEM], f32, tag="h64")
                    nc.vector.memset(h64[:, HID:], 0.0)
                    if ones_fast:
                        nc.vector.tensor_scalar(out=h64[:, :HID], in0=h_ps[:],
                                                scalar1=dinv_t[:, g:g + 1], scalar2=None,
                                                op0=mybir.AluOpType.mult)
                    else:
                        nc.vector.tensor_copy(out=h64[:, :HID], in_=h_ps[:])
                    nc.vector.tensor_copy(out=h_own[:, g, :], in_=h64[:, :HID])
                    nc.sync.dma_start(
                        out=in_bs[0][:].rearrange("(g p e) -> g p e", p=128, e=ELEM)[g],
                        in_=h64[:])

                def do_postproc(L, g, agg_ps):
                    A = wp.tile([128, HID], f32, tag="A")
                    if ones_fast:
                        # table rows are pre-scaled by dinv[src]; out =
                        # dinv[d]*(agg + h_own) + b
                        tmp = wp.tile([128, HID], f32, tag="tmp")
                        nc.vector.tensor_tensor(out=tmp[:], in0=agg_ps[:],
                                                in1=h_own[:, g, :],
                                                op=mybir.AluOpType.add)
                        nc.vector.tensor_scalar(
                            out=tmp[:], in0=tmp[:], scalar1=dinv_t[:, g:g + 1],
                            scalar2=None, op0=mybir.AluOpType.mult)
                        nc.vector.tensor_tensor(out=A[:], in0=tmp[:], in1=brep_t[:, L, :],
                                                op=mybir.AluOpType.add)
                    else:
                        nc.vector.tensor_tensor(out=A[:], in0=agg_ps[:], in1=brep_t[:, L, :],
                                                op=mybir.AluOpType.add)
                        tmp = wp.tile([128, HID], f32, tag="tmp")
                        nc.vector.tensor_scalar(
                            out=tmp[:], in0=h_own[:, g, :], scalar1=s2_t[:, g:g + 1],
                            scalar2=None, op0=mybir.AluOpType.mult)
                        nc.vector.tensor_tensor(out=A[:], in0=A[:], in1=tmp[:],
                                                op=mybir.AluOpType.add)
                    sq = wp.tile([128, HID], f32, tag="sq")
                    n2 = wp.tile([128, 1], f32, tag="n2")
                    nc.scalar.activation(out=sq[:], in_=A[:],
                                         func=mybir.ActivationFunctionType.Square,
                                         accum_out=n2[:])
                    nc.vector.tensor_scalar(out=n2[:], in0=n2[:], scalar1=EPS2,
                                            scalar2=None, op0=mybir.AluOpType.max)
                    rt_ = wp.tile([128, 1], f32, tag="rt")
                    nc.scalar.activation(out=rt_[:], in_=n2[:],
                                         func=mybir.ActivationFunctionType.Sqrt)
                    rinv = wp.tile([128, 1], f32, tag="rinv")
                    nc.vector.reciprocal(out=rinv[:], in_=rt_[:])
                    nc.scalar.activation(out=o_bufs[L][:, g, :], in_=A[:],
                                         func=mybir.ActivationFunctionType.Relu,
                                         scale=rinv[:])
                    if L < 2:
                        chain_write(L, g, o_bufs[L][:, g, :])
                    else:
                        # head fused into layer-3 postproc: y_g = sum_l o_l @ W_l + b
                        y_ps = pp.tile([128, NCLS], f32, tag="hps")
                        for Lh in range(3):
                            ot_ps = pp2.tile([HID, 128], f32, tag="tps")
                            nc.tensor.transpose(out=ot_ps[:], in_=o_bufs[Lh][:, g, :],
                                                identity=eye_t[:])
                            ot_sb = wp.tile([HID, 128], f32, tag="otsb")
                            nc.vector.tensor_copy(out=ot_sb[:], in_=ot_ps[:])
                            nc.tensor.matmul(y_ps[:], lhsT=ot_sb[:], rhs=wl_ts[Lh][:],
                                             start=(Lh == 0), stop=(Lh == 2),
                                             skip_group_check=True)
                        y_sb = wp.tile([128, NCLS], f32, tag="ysb")
                        nc.vector.tensor_tensor(out=y_sb[:], in0=y_ps[:], in1=blrep_t[:],
                                                op=mybir.AluOpType.add)
                        nc.sync.dma_start(out=y_view[g], in_=y_sb[:])

                pending = []
                y_view = t_y.rearrange("(g p k) -> g p k", p=128, k=NCLS)

                # ---- layers ----
                for L in range(3):
                    for q in range(4):
                        in_slice = in_bs[L][QB[q] * ELEM:QB[q + 1] * ELEM]
                        if not skip_collective:
                            nc.gpsimd.collective_compute(
                                "AllGather", mybir.AluOpType.bypass,
                                replica_groups=[list(range(NCORES))],
                                ins=[in_slice], outs=[qts[L][q][:]])
                        else:
                            # timing stand-in: 8 local copies emulate a perfect
                            # shared-memory allgather (correctness broken)
                            for c in range(NCORES):
                                nc.sync.dma_start(
                                    out=qts[L][q][SUBS[q] * ELEM * c:SUBS[q] * ELEM * (c + 1)],
                                    in_=in_slice)
                    tab_q = [qts[L][q][:].rearrange("(n e) -> n e", e=ELEM)
                             for q in range(4)]

                    # supergroup tile extents
                    sg_t0 = [min(t0 for (s, q, t0, nt) in call_meta if s == sg)
                             for sg in range(NSG)]
                    sg_t1 = [max(t0 + nt for (s, q, t0, nt) in call_meta if s == sg)
                             for sg in range(NSG)]

                    for sg in range(NSG):
                        t0s, t1s = sg_t0[sg], sg_t1[sg]
                        nts = t1s - t0s
                        msg = mp.tile([128, nts, ELEM], f32, tag="msg")
                        if skip_gather:
                            nc.gpsimd.memset(msg[:], 0.0)
                        for (s, q, t0, nt) in call_meta:
                            if s != sg or nt == 0:
                                continue
                            nidx = nt * 128
                            idxt = wp.tile([128, nt * 8], mybir.dt.int16, tag="idxt")
                            nc.sync.dma_start(
                                out=idxt[:],
                                in_=t_idx[t0 * 1024:t0 * 1024 + nidx * 8]
                                    .rearrange("(p n) -> p n", p=128))
                            if not skip_gather:
                                nc.gpsimd.dma_gather(
                                    out_ap=msg[:, t0 - t0s:t0 - t0s + nt, :],
                                    in_ap=tab_q[q],
                                    idxs_ap=idxt[:],
                                    num_idxs=nidx, num_idxs_reg=nidx,
                                    elem_size=ELEM, elem_step=ELEM,
                                    single_packet=False, queue_num=q)
                            if not ones_fast:
                                nc.vector.tensor_tensor(
                                    out=msg[:, t0 - t0s:t0 - t0s + nt, :HID],
                                    in0=msg[:, t0 - t0s:t0 - t0s + nt, :HID],
                                    in1=cnorm_t[:, t0:t0 + nt].unsqueeze(-1).to_broadcast([128, nt, HID]),
                                    op=mybir.AluOpType.mult)

                        # one psum group per 128-dst node group
                        for wi in range(SGW):
                            w_i = sg * SGW + wi
                            g = w_i
                            agg_ps = pp.tile([128, HID], f32, tag="agg")
                            if skip_scatter:
                                nc.vector.memset(agg_ps[:], 0.0)
                            tiles_w = []
                            for q in range(4):
                                for k in range(t_wq[w_i, q]):
                                    tiles_w.append(tile_of[(w_i, q, k)])
                            # S tiles in runs of up to 8 consecutive
                            runs = []
                            for t in tiles_w:
                                if runs and runs[-1][0] + runs[-1][1] == t and runs[-1][1] < 8:
                                    runs[-1] = (runs[-1][0], runs[-1][1] + 1)
                                else:
                                    runs.append((t, 1))
                            s_tiles = {}
                            for (rt, rn) in (runs if not skip_scatter else []):
                                st = spool.tile([128, 8, W], f32, tag="S")
                                nc.vector.tensor_tensor(
                                    out=st[:, :rn, :],
                                    in0=dstrel_t[:, rt:rt + rn].unsqueeze(-1).to_broadcast([128, rn, W]),
                                    in1=iota_t[:].unsqueeze(1).to_broadcast([128, rn, W]),
                                    op=mybir.AluOpType.is_equal)
                                for j in range(rn):
                                    s_tiles[rt + j] = st[:, j, :]
                            for ti, t in (enumerate(tiles_w) if not skip_scatter else []):
                                nc.tensor.matmul(
                                    agg_ps[:],
                                    lhsT=s_tiles[t],
                                    rhs=msg[:, t - t0s, :HID],
                                    start=(ti == 0), stop=(ti == len(tiles_w) - 1),
                                    skip_group_check=True)
                            pending.append((g, agg_ps))
                            if len(pending) > 2:
                                do_postproc(L, *pending.pop(0))
                    for item in pending:
                        do_postproc(L, *item)
                    pending.clear()

    nc.compile()
    return nc


def _make_in_maps(meta, W1, b1, W2, b2, W3, b3, W_lin, b_lin):
    brep = np.stack([np.tile(np.asarray(b, np.float32)[None, :], (128, 1))
                     for b in (b1, b2, b3)])           # [3,128,HID]
    blrep = np.tile(np.asarray(b_lin, np.float32)[None, :], (128, 1))
    iota = np.tile(np.arange(W, dtype=np.float32)[None, :], (128, 1))
    eye = np.eye(128, dtype=np.float32)
    maps = []
    for c in range(NCORES):
        maps.append({
            "x_c": meta["x_pad"][c].reshape(-1),
            "idxb": meta["idx_blocks"][c],
            "cnorm": meta["cnorm_pt"][c],
            "dstrel": meta["dstrel_pt"][c],
            "s2": meta["s2_pt"][c],
            "dinv": meta["dinv_pt"][c],
            "w1": np.asarray(W1, np.float32), "w2": np.asarray(W2, np.float32),
            "w3": np.asarray(W3, np.float32), "wl": np.asarray(W_lin, np.float32),
            "brep": brep, "blrep": blrep, "iota": iota, "eye": eye,
        })
    return maps


def kernel(x, edge_index, edge_weights, W1, b1, W2, b2, W3, b3, W_lin, b_lin):
    meta = _host_prep(x, edge_index, edge_weights)
    key = ("prog", meta["ones_fast"])
    if key not in _cache:
        _cache[key] = _build_program(meta, ones_fast=meta["ones_fast"])
    nc = _cache[key]
    in_maps = _make_in_maps(meta, W1, b1, W2, b2, W3, b3, W_lin, b_lin)
    res = run_bass_kernel_spmd(nc, in_maps, core_ids=list(range(NCORES)))
    ys = [res.results[c]["y"].reshape(PC, NCLS)[:RC] for c in range(NCORES)]
    return np.concatenate(ys, axis=0).astype(np.float32)

